# revision 62
# baseline (speedup 1.0000x reference)
"""Tensor-parallel GQA attention block (AtlasAttentionWrapper) on 8 TRN2 cores.

Sharding: TP over heads. Core m owns query heads [4m..4m+3] (Wq rows
m*512:(m+1)*512), KV head m (Wk/Wv rows m*128:(m+1)*128, past_k/past_v head m)
and Wo columns m*512:(m+1)*512. Each core computes a full [1024, 4096] o_proj
partial; chunked ReduceScatters ([512, 256, 256] rows, pipelined under the
remaining attention/o_proj compute) leave each core 1/8 of the rows of each
chunk; the host reassembles.

All device inputs are host-packed into SBUF layout [128, chunks, inner] so
every DMA is a large transfer with long contiguous per-partition lines.

Phase 1 runs chunk-outer with persistent PSUM accumulators so the PE starts
as soon as the first weight/xT pieces land:
  pass A  accumulates K, V, Q0 (6 banks) over all 32 contraction chunks,
  passes B1..B3 accumulate Q1..Q3 (2 banks each), reusing banks whose
  consumers (V copy, K rope, Q0 rope) have already drained.
Each rope is emitted right after its producing pass so DVE overlaps the next
pass's matmuls; per-head qT tiles keep attention from waiting on later ropes.
Attention (scores built transposed, exp on scalar with 2-chunk batching,
softmax denominator via ones-matmul) and o_proj + chunked ReduceScatter are
interleaved: attn(g0), oproj k0/k1 + RS, attn(g1), oproj k2/k3 + RS.
"""

import sys

if "/opt/trn_rl_repo" not in sys.path:
    sys.path.insert(0, "/opt/trn_rl_repo")

from contextlib import ExitStack

import ml_dtypes
import numpy as np

import concourse.bass as bass
import concourse.tile as tile
from concourse import bacc, mybir
from concourse.bass import ds, ts
from concourse.bass_utils import run_bass_kernel_spmd
from concourse.masks import make_identity

NCORES = 8
B, SQ, H = 1, 1024, 4096
NH, NKV, D = 32, 8, 128
SP = 1024
KV = SP + SQ  # 2048
HPC = NH // NCORES  # 4 query heads per core
DQ = HPC * D  # 512
SH = SQ // NCORES  # 128 output rows per core after ReduceScatter
ROPE_THETA = 10000.0
INV_SQRT_D = 1.0 / float(np.sqrt(D))

BF16 = mybir.dt.bfloat16
F32 = mybir.dt.float32
HCH = H // 128  # 32 contraction chunks
KVCH = KV // 128  # 16 kv chunks
# ReduceScatter chunk sizes (rows), matching the attention column groups:
# early 2MB chunks start the (HBM-bound, continuously-busy) collective
# stream as soon as possible; the final narrow group + 1MB chunk pulls the
# last, fully-exposed RS forward.
CHUNK_ROWS = [256, 256, 384, 128]
CHUNK_OFF = [0, 256, 512, 896]
NCHUNK = len(CHUNK_ROWS)
EXP = mybir.ActivationFunctionType.Exp

LAST_RESULT = None
_NC_CACHE = {}


def _rope_write(nc, tmp_pool, dst, src, cos_sb, sin_sb, pos, width):
    """dst[d, s] = rope(src)[d, s] for s in [pos, pos+width) absolute positions.

    src: AP [128, width] (PSUM f32 or SBUF bf16), dst: SBUF bf16 AP.
    rope: out[d<64] = x[d]*cos[d] - x[d+64]*sin[d]
          out[d>=64] = x[d]*cos[d] + x[d-64]*sin[d]
    """
    cs = cos_sb[:, ds(pos, width)]
    sn = sin_sb[:, ds(pos, width)]
    t = tmp_pool.tile([128, width], F32, tag="rope_t")
    u = tmp_pool.tile([128, width], F32, tag="rope_u")
    nc.vector.tensor_mul(t[0:64, :], src[64:128, :], sn[0:64, :])
    nc.vector.tensor_mul(t[64:128, :], src[0:64, :], sn[64:128, :])
    nc.vector.tensor_mul(u[:, :], src[:, :], cs)
    nc.vector.tensor_sub(dst[0:64, :], u[0:64, :], t[0:64, :])
    nc.vector.tensor_add(dst[64:128, :], u[64:128, :], t[64:128, :])


def _build_nc():
    nc = bacc.Bacc(None, target_bir_lowering=False, debug=False)

    xTp = nc.declare_dram_parameter("xTp", [128, HCH, SQ], BF16, False)
    wqp = nc.declare_dram_parameter("wqp", [128, HCH, DQ], BF16, False)
    wkp = nc.declare_dram_parameter("wkp", [128, HCH, D], BF16, False)
    wvp = nc.declare_dram_parameter("wvp", [128, HCH, D], BF16, False)
    wop = nc.declare_dram_parameter("wop", [128, HPC, H], BF16, False)
    pkT = nc.declare_dram_parameter("pkT", [D, SP], BF16, False)
    pvp = nc.declare_dram_parameter("pvp", [128, SP // 128, D], BF16, False)
    cosk = nc.declare_dram_parameter("cosk", [D, KV], BF16, False)
    sink = nc.declare_dram_parameter("sink", [D, KV], BF16, False)
    out_ext = nc.declare_dram_parameter("out", [SH, H], BF16, True)

    with tile.TileContext(nc) as tc, ExitStack() as ctx:
        # ---- persistent SBUF residents (live across all phases)
        const = ctx.enter_context(tc.tile_pool(name="const", bufs=1))
        kT_sb = const.tile([128, KV], BF16)  # roped K^T  [d, kv]
        v_sb = const.tile([128, KVCH, D], BF16)  # V chunks [kv%128, chunk, d]
        # per-head roped Q^T / attn^T tiles (separate tiles keep readers from
        # waiting on later heads' writes)
        qT = [const.tile([128, SQ], BF16, name=f"qT{j}") for j in range(HPC)]
        attnT = [const.tile([128, SQ], BF16, name=f"attnT{j}") for j in range(HPC)]
        cos_sb = const.tile([128, KV], BF16)
        sin_sb = const.tile([128, KV], BF16)
        ident = const.tile([128, 128], BF16)
        ones_sb = const.tile([128, 128], BF16)

        make_identity(nc, ident[:, :])
        nc.vector.memset(ones_sb[:, :], 1.0)

        rope_tmp = ctx.enter_context(tc.tile_pool(name="rope_tmp", bufs=2))
        dram = ctx.enter_context(tc.tile_pool(name="dram", bufs=1, space="DRAM"))
        part_chunks = []
        rs_chunks = []
        for k in range(NCHUNK):
            part_chunks.append(
                dram.tile([CHUNK_ROWS[k], H], BF16, tag=f"part{k}", name=f"part{k}")
            )
            rs_chunks.append(
                dram.tile(
                    [CHUNK_ROWS[k] // NCORES, H], BF16, tag=f"rs{k}", name=f"rs{k}"
                )
            )

        # ================= Phase 1: projections + rope ==================
        with tc.tile_pool(name="proj", bufs=1) as proj, tc.tile_pool(
            name="ph1_ps", bufs=1, space="PSUM"
        ) as ph1_ps:
            xT_sb = proj.tile([128, HCH, SQ], BF16)
            wqT_sb = proj.tile([128, HCH, DQ], BF16)
            wkT_sb = proj.tile([128, HCH, D], BF16)
            wvT_sb = proj.tile([128, HCH, D], BF16)

            # Weight/xT stream: small first pieces so the chunk-outer pass A
            # starts within a few us; later pieces sized to stay ahead of PE.
            nc.sync.dma_start(out=wkT_sb[:, 0:2, :], in_=wkp[:, 0:2, :])
            nc.sync.dma_start(out=xT_sb[:, 0:1, :], in_=xTp[:, 0:1, :])
            nc.sync.dma_start(out=wvT_sb[:, 0:2, :], in_=wvp[:, 0:2, :])
            nc.sync.dma_start(out=xT_sb[:, 1:2, :], in_=xTp[:, 1:2, :])
            nc.sync.dma_start(out=wkT_sb[:, 2:8, :], in_=wkp[:, 2:8, :])
            nc.sync.dma_start(out=wvT_sb[:, 2:8, :], in_=wvp[:, 2:8, :])
            nc.sync.dma_start(out=xT_sb[:, 2:4, :], in_=xTp[:, 2:4, :])
            nc.sync.dma_start(out=xT_sb[:, 4:6, :], in_=xTp[:, 4:6, :])
            nc.sync.dma_start(out=wkT_sb[:, 8:16, :], in_=wkp[:, 8:16, :])
            nc.sync.dma_start(out=wvT_sb[:, 8:16, :], in_=wvp[:, 8:16, :])
            nc.sync.dma_start(out=xT_sb[:, 6:8, :], in_=xTp[:, 6:8, :])
            nc.sync.dma_start(out=xT_sb[:, 8:12, :], in_=xTp[:, 8:12, :])
            nc.sync.dma_start(out=xT_sb[:, 12:16, :], in_=xTp[:, 12:16, :])
            nc.sync.dma_start(out=wkT_sb[:, 16:32, :], in_=wkp[:, 16:32, :])
            nc.sync.dma_start(out=wvT_sb[:, 16:32, :], in_=wvp[:, 16:32, :])
            nc.sync.dma_start(out=xT_sb[:, 16:20, :], in_=xTp[:, 16:20, :])
            nc.sync.dma_start(out=xT_sb[:, 20:24, :], in_=xTp[:, 20:24, :])
            nc.sync.dma_start(out=xT_sb[:, 24:32, :], in_=xTp[:, 24:32, :])
            # scalar queue: Q weights in consumption order (chunks >= QDEFER
            # are needed first; chunks 0:8 only at the end-of-A mini-sweep),
            # then the small rope/past tensors (first needed at the K rope).
            nc.scalar.dma_start(out=wqT_sb[:, 8:16, :], in_=wqp[:, 8:16, :])
            nc.scalar.dma_start(out=wqT_sb[:, 16:24, :], in_=wqp[:, 16:24, :])
            nc.scalar.dma_start(out=wqT_sb[:, 24:32, :], in_=wqp[:, 24:32, :])
            nc.scalar.dma_start(out=wqT_sb[:, 0:8, :], in_=wqp[:, 0:8, :])
            nc.scalar.dma_start(out=cos_sb[:, :], in_=cosk[:, :])
            nc.scalar.dma_start(out=sin_sb[:, :], in_=sink[:, :])
            nc.scalar.dma_start(out=v_sb[:, 0 : SP // 128, :], in_=pvp[:, :, :])
            nc.scalar.dma_start(out=kT_sb[:, 0:SP], in_=pkT[:, :])

            # ---- pass A: chunk-outer accumulation of K, V, Q0, Q1 (8 banks).
            # The first QDEFER chunks contribute only K/V (light, 2.1us/chunk)
            # so the PE never catches the still-warming DMA stream; their
            # Q0/Q1 contributions run as a mini-sweep at the end of the pass
            # (accumulation order within a bank is free).
            QDEFER = 8
            kps = ph1_ps.tile([128, 2, 512], F32, tag="pa0")
            vps = ph1_ps.tile([128, 2, 512], F32, tag="pa1")
            q0ps = ph1_ps.tile([128, 2, 512], F32, tag="pa2")
            q1ps = ph1_ps.tile([128, 2, 512], F32, tag="pa3")

            def q01_mms(c, st_flags):
                for g in range(2):
                    nc.tensor.matmul(
                        q0ps[:, g, :],
                        lhsT=wqT_sb[:, c, 0:128],
                        rhs=xT_sb[:, c, ts(g, 512)],
                        **st_flags,
                    )
                    nc.tensor.matmul(
                        q1ps[:, g, :],
                        lhsT=wqT_sb[:, c, ds(128, 128)],
                        rhs=xT_sb[:, c, ts(g, 512)],
                        **st_flags,
                    )

            for c in range(HCH):
                st_flags = dict(start=(c == 0), stop=(c == HCH - 1))
                for g in range(2):
                    nc.tensor.matmul(
                        kps[:, g, :],
                        lhsT=wkT_sb[:, c, :],
                        rhs=xT_sb[:, c, ts(g, 512)],
                        **st_flags,
                    )
                    nc.tensor.matmul(
                        vps[:, g, :],
                        lhsT=wvT_sb[:, c, :],
                        rhs=xT_sb[:, c, ts(g, 512)],
                        **st_flags,
                    )
                if c >= QDEFER:
                    q01_mms(c, dict(start=(c == QDEFER), stop=False))
            for c in range(QDEFER):
                q01_mms(c, dict(start=False, stop=(c == QDEFER - 1)))

            # V psum -> bf16 staging (scalar; fast consumer frees pa1)
            vt_sb = proj.tile([128, 2, 512], BF16)
            nc.scalar.activation(
                vt_sb[:, :, :], vps[:, :, :], mybir.ActivationFunctionType.Copy
            )
            # K + Q0 + Q1 ropes on DVE (overlap the transposes / pass B2)
            for g in range(2):
                _rope_write(
                    nc, rope_tmp, kT_sb[:, ds(SP + g * 512, 512)], kps[:, g, :],
                    cos_sb, sin_sb, SP + g * 512, 512,
                )
            for g in range(2):
                _rope_write(
                    nc, rope_tmp, qT[0][:, ts(g, 512)], q0ps[:, g, :],
                    cos_sb, sin_sb, SP + g * 512, 512,
                )
            for g in range(2):
                _rope_write(
                    nc, rope_tmp, qT[1][:, ts(g, 512)], q1ps[:, g, :],
                    cos_sb, sin_sb, SP + g * 512, 512,
                )
            # V transposes into v_sb chunks [SP/128 ..): reuse pa1's banks
            # ([128,128] bf16 tiles in the slot vt's copy just freed)
            for k in range(8):
                ps2 = ph1_ps.tile([128, 128], BF16, tag="pa1", name="trps")
                nc.tensor.transpose(
                    ps2[:, :], vt_sb[:, k // 4, ts(k % 4, 128)], ident[:, :]
                )
                nc.scalar.copy(v_sb[:, SP // 128 + k, :], ps2[:, :])

            # ---- passes B2/B3: Q2 (reuses K's banks), Q3
            def q_pass(j, tag):
                qps = ph1_ps.tile([128, 2, 512], F32, tag=tag, name=f"q{j}ps")
                for c in range(HCH):
                    st_flags = dict(start=(c == 0), stop=(c == HCH - 1))
                    for g in range(2):
                        nc.tensor.matmul(
                            qps[:, g, :],
                            lhsT=wqT_sb[:, c, ts(j, 128)],
                            rhs=xT_sb[:, c, ts(g, 512)],
                            **st_flags,
                        )
                return qps

            q2ps = q_pass(2, "pa0")
            for g in range(2):
                _rope_write(
                    nc, rope_tmp, qT[2][:, ts(g, 512)], q2ps[:, g, :],
                    cos_sb, sin_sb, SP + g * 512, 512,
                )
            # B3 runs g-outer so the g0 bank's accumulation finishes mid-pass;
            # its PSUM-reading rope muls (stage 1) then overlap the g1 loop,
            # freeing B3's banks (aliased by the attention pools) early.
            q3ps = ph1_ps.tile([128, 2, 512], F32, tag="pa2", name="q3ps")
            q3t, q3u = [], []
            for g in range(2):
                for c in range(HCH):
                    nc.tensor.matmul(
                        q3ps[:, g, :],
                        lhsT=wqT_sb[:, c, ts(3, 128)],
                        rhs=xT_sb[:, c, ts(g, 512)],
                        start=(c == 0),
                        stop=(c == HCH - 1),
                    )
                cs = cos_sb[:, ds(SP + g * 512, 512)]
                sn = sin_sb[:, ds(SP + g * 512, 512)]
                t = rope_tmp.tile([128, 512], F32, tag="rope_t")
                u = rope_tmp.tile([128, 512], F32, tag="rope_u")
                nc.vector.tensor_mul(t[0:64, :], q3ps[64:128, g, :], sn[0:64, :])
                nc.vector.tensor_mul(t[64:128, :], q3ps[0:64, g, :], sn[64:128, :])
                nc.vector.tensor_mul(u[:, :], q3ps[:, g, :], cs)
                q3t.append(t)
                q3u.append(u)
            for g in range(2):
                dst = qT[3][:, ts(g, 512)]
                nc.vector.tensor_sub(dst[0:64, :], q3u[g][0:64, :], q3t[g][0:64, :])
                nc.vector.tensor_add(
                    dst[64:128, :], q3u[g][64:128, :], q3t[g][64:128, :]
                )

        # ============ Phase 2+3 interleaved: attention, o_proj, RS ==========
        # separate pools: scores (2 pairs in flight), o_proj accumulators
        # (double-buffered), and single-buffered sums/att (their consumers
        # drain within the next head's lead-in) — 4+2+1+1 = 8 banks
        st_ps = ctx.enter_context(tc.tile_pool(name="st_ps", bufs=2, space="PSUM"))
        ops_ps = ctx.enter_context(tc.tile_pool(name="ops_ps", bufs=2, space="PSUM"))
        sums_ps = ctx.enter_context(
            tc.tile_pool(name="sums_ps", bufs=1, space="PSUM")
        )
        at_ps = ctx.enter_context(tc.tile_pool(name="at_ps", bufs=1, space="PSUM"))

        pt_pool = ctx.enter_context(tc.tile_pool(name="pt", bufs=6))
        pa_pool = ctx.enter_context(tc.tile_pool(name="pa", bufs=4))
        rc_pool = ctx.enter_context(tc.tile_pool(name="rc", bufs=2))
        wo_pool = ctx.enter_context(tc.tile_pool(name="wo", bufs=1))
        # deep o_proj staging: a full RS chunk (2MB) fits in SBUF so the PE
        # never stalls on part-chunk DMA writes slowed by a concurrent RS
        ob_pool = ctx.enter_context(tc.tile_pool(name="ob", bufs=16))
        wo_sb = wo_pool.tile([128, HPC, H], BF16)
        nc.scalar.dma_start(out=wo_sb[:, :, :], in_=wop[:, :, :])



        def attention_cols(off, w):
            for h in range(HPC):
                sums = sums_ps.tile([128, 512], F32, tag="sums", name=f"s{h}{off}")
                att = at_ps.tile([128, 512], F32, tag="att", name=f"a{h}{off}")
                for cc in range(KVCH // 2):
                    st = st_ps.tile([128, 2, 512], F32, tag="st", name="st")
                    pt = pt_pool.tile([128, 2, 512], BF16, name="pt")
                    for j in range(2):
                        nc.tensor.matmul(
                            st[:, j, 0:w],
                            lhsT=kT_sb[:, ts(2 * cc + j, 128)],
                            rhs=qT[h][:, ds(off, w)],
                            start=True,
                            stop=True,
                        )
                    nc.scalar.activation(
                        pt[:, :, 0:w], st[:, :, 0:w], EXP, scale=INV_SQRT_D
                    )
                    # softmax denominator: DVE pre-sums the chunk pair (bf16),
                    # halving the PE's ones-matmul work
                    padd = pa_pool.tile([128, 512], BF16, name="padd")
                    nc.vector.tensor_add(
                        padd[:, 0:w], pt[:, 0, 0:w], pt[:, 1, 0:w]
                    )
                    nc.tensor.matmul(
                        sums[:, 0:w],
                        lhsT=ones_sb[:, :],
                        rhs=padd[:, 0:w],
                        start=(cc == 0),
                        stop=(cc == KVCH // 2 - 1),
                    )
                    for j in range(2):
                        c = 2 * cc + j
                        nc.tensor.matmul(
                            att[:, 0:w],
                            lhsT=v_sb[:, c, :],
                            rhs=pt[:, j, 0:w],
                            start=(c == 0),
                            stop=(c == KVCH - 1),
                        )
                recip = rc_pool.tile([128, 512], F32, name="recip")
                nc.vector.reciprocal_approx_fast(recip[:, 0:w], sums[:, 0:w])
                nc.vector.tensor_mul(
                    attnT[h][:, ds(off, w)], att[:, 0:w], recip[:, 0:w]
                )

        def oproj_chunk(k):
            for ii in range(CHUNK_ROWS[k] // 128):
                i = CHUNK_OFF[k] // 128 + ii
                for n in range(H // 512):
                    ps = ops_ps.tile([128, 512], F32, tag="ops", name="ops")
                    ob = ob_pool.tile([128, 512], BF16, name="ob")
                    for j in range(HPC):
                        nc.tensor.matmul(
                            ps[:, :],
                            lhsT=attnT[j][:, ts(i, 128)],
                            rhs=wo_sb[:, j, ts(n, 512)],
                            start=(j == 0),
                            stop=(j == HPC - 1),
                        )
                    nc.vector.tensor_copy(ob[:, :], ps[:, :])
                    # last chunk's writes go on the scalar queue (idle by
                    # then) so they spread across descriptor queues while
                    # racing the previous chunk's ReduceScatter
                    dmaq = nc.scalar if k == NCHUNK - 1 else nc.sync
                    dmaq.dma_start(
                        out=part_chunks[k][ts(ii, 128), ts(n, 512)],
                        in_=ob[:, :],
                    )
            nc.gpsimd.collective_compute(
                "ReduceScatter",
                mybir.AluOpType.add,
                ins=[part_chunks[k][:, :].opt()],
                outs=[rs_chunks[k][:, :].opt()],
                replica_groups=[list(range(NCORES))],
            )
            # gpsimd queue: an RS-gated trigger here can't block the sync
            # queue's part writes or the scalar queue's attention exps
            nc.gpsimd.dma_start(
                out=out_ext[ds(CHUNK_OFF[k] // NCORES, CHUNK_ROWS[k] // NCORES), :],
                in_=rs_chunks[k][:, :],
            )

        # attention in four 256-wide passes, each immediately followed by its
        # o_proj chunk + ReduceScatter: every RS overlaps the next group's
        # compute, and the collective stream starts ~50us earlier than with
        # 512-wide halves.
        attention_cols(0, 256)
        oproj_chunk(0)
        attention_cols(256, 256)
        oproj_chunk(1)
        attention_cols(512, 384)
        oproj_chunk(2)
        attention_cols(896, 128)
        oproj_chunk(3)

    nc.finalize()
    return nc


def _get_nc():
    if "nc" not in _NC_CACHE:
        _NC_CACHE["nc"] = _build_nc()
    return _NC_CACHE["nc"]


def _rope_tables():
    inv_freq = 1.0 / (ROPE_THETA ** (np.arange(0, D, 2, dtype=np.float32) / D))
    pos = np.arange(KV, dtype=np.float32)
    freqs = pos[:, None] * inv_freq[None, :]  # [KV, D/2]
    emb = np.concatenate([freqs, freqs], axis=-1)  # [KV, D]
    return np.cos(emb), np.sin(emb)  # [KV, D]


def _host_rope(x, cos, sin):
    # x: [S, D]; cos/sin: [S, D]
    x1, x2 = x[:, : D // 2], x[:, D // 2 :]
    rot = np.concatenate([-x2, x1], axis=-1)
    return x * cos + rot * sin


def _pack(mat_t, inner):
    """[n*128, inner] -> [128, n, inner]: SBUF layout, partition dim first."""
    n = mat_t.shape[0] // 128
    return np.ascontiguousarray(mat_t.reshape(n, 128, inner).transpose(1, 0, 2))


def kernel(hidden_states, past_k, past_v, Wq, Wk, Wv, Wo, trace=False):
    global LAST_RESULT
    bf = ml_dtypes.bfloat16
    x = np.asarray(hidden_states, dtype=np.float32)[0]  # [SQ, H]
    xTp = _pack(np.ascontiguousarray(x.T), SQ).astype(bf)
    cos, sin = _rope_tables()  # [KV, D] f32
    cosT = np.ascontiguousarray(cos.T).astype(bf)
    sinT = np.ascontiguousarray(sin.T).astype(bf)

    in_maps = []
    for m in range(NCORES):
        qr = slice(m * DQ, (m + 1) * DQ)
        kr = slice(m * D, (m + 1) * D)
        in_maps.append(
            {
                "xTp": xTp,
                "wqp": _pack(np.asarray(Wq)[qr].T, DQ).astype(bf),
                "wkp": _pack(np.asarray(Wk)[kr].T, D).astype(bf),
                "wvp": _pack(np.asarray(Wv)[kr].T, D).astype(bf),
                "wop": _pack(np.asarray(Wo)[:, qr].T, H).astype(bf),
                "pkT": np.ascontiguousarray(
                    _host_rope(
                        np.asarray(past_k, dtype=np.float32)[0, m], cos[:SP], sin[:SP]
                    ).T
                ).astype(bf),
                "pvp": _pack(np.asarray(past_v)[0, m], D).astype(bf),
                "cosk": cosT,
                "sink": sinT,
            }
        )

    nc = _get_nc()
    res = run_bass_kernel_spmd(
        nc, in_maps, core_ids=list(range(NCORES)), trace=trace
    )
    LAST_RESULT = res
    # Each core's "out" holds NCHUNK blocks of CHUNK_ROWS[k]/8 rows; block k
    # of core m is global rows CHUNK_OFF[k] + rsh_k*[m, m+1).
    out = np.empty((SQ, H), dtype=np.float32)
    for m in range(NCORES):
        shard = np.asarray(res.results[m]["out"], dtype=np.float32)
        for k in range(NCHUNK):
            rsh = CHUNK_ROWS[k] // NCORES
            soff = CHUNK_OFF[k] // NCORES
            out[CHUNK_OFF[k] + rsh * m : CHUNK_OFF[k] + rsh * (m + 1)] = shard[
                soff : soff + rsh
            ]
    return out.reshape(B, SQ, H)


# revision 64
# speedup vs baseline: 1.1020x; 1.1020x over previous
"""Tensor-parallel GQA attention block (AtlasAttentionWrapper) on 8 TRN2 cores.

Sharding: TP over heads. Core m owns query heads [4m..4m+3] (Wq rows
m*512:(m+1)*512), KV head m (Wk/Wv rows m*128:(m+1)*128, past_k/past_v head m)
and Wo columns m*512:(m+1)*512. Each core computes a full [1024, 4096] o_proj
partial; chunked ReduceScatters ([512, 256, 256] rows, pipelined under the
remaining attention/o_proj compute) leave each core 1/8 of the rows of each
chunk; the host reassembles.

All device inputs are host-packed into SBUF layout [128, chunks, inner] so
every DMA is a large transfer with long contiguous per-partition lines.

Phase 1 runs chunk-outer with persistent PSUM accumulators so the PE starts
as soon as the first weight/xT pieces land:
  pass A  accumulates K, V, Q0 (6 banks) over all 32 contraction chunks,
  passes B1..B3 accumulate Q1..Q3 (2 banks each), reusing banks whose
  consumers (V copy, K rope, Q0 rope) have already drained.
Each rope is emitted right after its producing pass so DVE overlaps the next
pass's matmuls; per-head qT tiles keep attention from waiting on later ropes.
Attention (scores built transposed, exp on scalar with 2-chunk batching,
softmax denominator via ones-matmul) and o_proj + chunked ReduceScatter are
interleaved: attn(g0), oproj k0/k1 + RS, attn(g1), oproj k2/k3 + RS.
"""

import sys

if "/opt/trn_rl_repo" not in sys.path:
    sys.path.insert(0, "/opt/trn_rl_repo")

from contextlib import ExitStack

import ml_dtypes
import numpy as np

import concourse.bass as bass
import concourse.tile as tile
from concourse import bacc, mybir
from concourse.bass import ds, ts
from concourse.bass_utils import run_bass_kernel_spmd
from concourse.masks import make_identity

NCORES = 8
B, SQ, H = 1, 1024, 4096
NH, NKV, D = 32, 8, 128
SP = 1024
KV = SP + SQ  # 2048
HPC = NH // NCORES  # 4 query heads per core
DQ = HPC * D  # 512
SH = SQ // NCORES  # 128 output rows per core after ReduceScatter
ROPE_THETA = 10000.0
INV_SQRT_D = 1.0 / float(np.sqrt(D))

BF16 = mybir.dt.bfloat16
F32 = mybir.dt.float32
HCH = H // 128  # 32 contraction chunks
KVCH = KV // 128  # 16 kv chunks
# ReduceScatter chunk sizes (rows), matching the attention column groups:
# early 2MB chunks start the (HBM-bound, continuously-busy) collective
# stream as soon as possible; the final narrow group + 1MB chunk pulls the
# last, fully-exposed RS forward.
CHUNK_ROWS = [256, 256, 256, 256]
CHUNK_OFF = [0, 256, 512, 768]
NCHUNK = len(CHUNK_ROWS)
EXP = mybir.ActivationFunctionType.Exp

LAST_RESULT = None
_NC_CACHE = {}


def _rope_write(nc, tmp_pool, dst, src, cos_sb, sin_sb, pos, width):
    """dst[d, s] = rope(src)[d, s] for s in [pos, pos+width) absolute positions.

    src: AP [128, width] (PSUM f32 or SBUF bf16), dst: SBUF bf16 AP.
    rope: out[d<64] = x[d]*cos[d] - x[d+64]*sin[d]
          out[d>=64] = x[d]*cos[d] + x[d-64]*sin[d]
    """
    cs = cos_sb[:, ds(pos, width)]
    sn = sin_sb[:, ds(pos, width)]
    t = tmp_pool.tile([128, width], F32, tag="rope_t")
    u = tmp_pool.tile([128, width], F32, tag="rope_u")
    nc.vector.tensor_mul(t[0:64, :], src[64:128, :], sn[0:64, :])
    nc.vector.tensor_mul(t[64:128, :], src[0:64, :], sn[64:128, :])
    nc.vector.tensor_mul(u[:, :], src[:, :], cs)
    nc.vector.tensor_sub(dst[0:64, :], u[0:64, :], t[0:64, :])
    nc.vector.tensor_add(dst[64:128, :], u[64:128, :], t[64:128, :])


def _build_nc():
    nc = bacc.Bacc(None, target_bir_lowering=False, debug=False)

    xTp = nc.declare_dram_parameter("xTp", [128, HCH, SQ], BF16, False)
    wqp = nc.declare_dram_parameter("wqp", [128, HCH, DQ], BF16, False)
    wkp = nc.declare_dram_parameter("wkp", [128, HCH, D], BF16, False)
    wvp = nc.declare_dram_parameter("wvp", [128, HCH, D], BF16, False)
    wop = nc.declare_dram_parameter("wop", [128, HPC, H], BF16, False)
    pkT = nc.declare_dram_parameter("pkT", [D, SP], BF16, False)
    pvp = nc.declare_dram_parameter("pvp", [128, SP // 128, D], BF16, False)
    cosk = nc.declare_dram_parameter("cosk", [D, KV], BF16, False)
    sink = nc.declare_dram_parameter("sink", [D, KV], BF16, False)
    out_ext = nc.declare_dram_parameter("out", [SH, H], BF16, True)

    with tile.TileContext(nc) as tc, ExitStack() as ctx:
        # ---- persistent SBUF residents (live across all phases)
        const = ctx.enter_context(tc.tile_pool(name="const", bufs=1))
        kT_sb = const.tile([128, KV], BF16)  # roped K^T  [d, kv]
        v_sb = const.tile([128, KVCH, D], BF16)  # V chunks [kv%128, chunk, d]
        # per-head roped Q^T / attn^T tiles (separate tiles keep readers from
        # waiting on later heads' writes)
        qT = [const.tile([128, SQ], BF16, name=f"qT{j}") for j in range(HPC)]
        attnT = [const.tile([128, SQ], BF16, name=f"attnT{j}") for j in range(HPC)]
        cos_sb = const.tile([128, KV], BF16)
        sin_sb = const.tile([128, KV], BF16)
        ident = const.tile([128, 128], BF16)
        ones_sb = const.tile([128, 128], BF16)

        make_identity(nc, ident[:, :])
        nc.vector.memset(ones_sb[:, :], 1.0)

        rope_tmp = ctx.enter_context(tc.tile_pool(name="rope_tmp", bufs=2))
        dram = ctx.enter_context(tc.tile_pool(name="dram", bufs=1, space="DRAM"))
        part_chunks = []
        rs_chunks = []
        for k in range(NCHUNK):
            part_chunks.append(
                dram.tile([CHUNK_ROWS[k], H], BF16, tag=f"part{k}", name=f"part{k}")
            )
            rs_chunks.append(
                dram.tile(
                    [CHUNK_ROWS[k] // NCORES, H], BF16, tag=f"rs{k}", name=f"rs{k}"
                )
            )

        # ================= Phase 1: projections + rope ==================
        with tc.tile_pool(name="proj", bufs=1) as proj, tc.tile_pool(
            name="ph1_ps", bufs=1, space="PSUM"
        ) as ph1_ps:
            xT_sb = proj.tile([128, HCH, SQ], BF16)
            wqT_sb = proj.tile([128, HCH, DQ], BF16)
            wkT_sb = proj.tile([128, HCH, D], BF16)
            wvT_sb = proj.tile([128, HCH, D], BF16)

            # Weight/xT stream: small first pieces so the chunk-outer pass A
            # starts within a few us; later pieces sized to stay ahead of PE.
            nc.sync.dma_start(out=wkT_sb[:, 0:2, :], in_=wkp[:, 0:2, :])
            nc.sync.dma_start(out=xT_sb[:, 0:1, :], in_=xTp[:, 0:1, :])
            nc.sync.dma_start(out=wvT_sb[:, 0:2, :], in_=wvp[:, 0:2, :])
            nc.sync.dma_start(out=xT_sb[:, 1:2, :], in_=xTp[:, 1:2, :])
            nc.sync.dma_start(out=wkT_sb[:, 2:8, :], in_=wkp[:, 2:8, :])
            nc.sync.dma_start(out=wvT_sb[:, 2:8, :], in_=wvp[:, 2:8, :])
            nc.sync.dma_start(out=xT_sb[:, 2:4, :], in_=xTp[:, 2:4, :])
            nc.sync.dma_start(out=xT_sb[:, 4:6, :], in_=xTp[:, 4:6, :])
            nc.sync.dma_start(out=wkT_sb[:, 8:16, :], in_=wkp[:, 8:16, :])
            nc.sync.dma_start(out=wvT_sb[:, 8:16, :], in_=wvp[:, 8:16, :])
            nc.sync.dma_start(out=xT_sb[:, 6:8, :], in_=xTp[:, 6:8, :])
            nc.sync.dma_start(out=xT_sb[:, 8:12, :], in_=xTp[:, 8:12, :])
            nc.sync.dma_start(out=xT_sb[:, 12:16, :], in_=xTp[:, 12:16, :])
            nc.sync.dma_start(out=wkT_sb[:, 16:32, :], in_=wkp[:, 16:32, :])
            nc.sync.dma_start(out=wvT_sb[:, 16:32, :], in_=wvp[:, 16:32, :])
            nc.sync.dma_start(out=xT_sb[:, 16:20, :], in_=xTp[:, 16:20, :])
            nc.sync.dma_start(out=xT_sb[:, 20:24, :], in_=xTp[:, 20:24, :])
            nc.sync.dma_start(out=xT_sb[:, 24:32, :], in_=xTp[:, 24:32, :])
            # scalar queue: Q weights in consumption order (chunks >= QDEFER
            # are needed first; chunks 0:8 only at the end-of-A mini-sweep),
            # then the small rope/past tensors (first needed at the K rope).
            nc.scalar.dma_start(out=wqT_sb[:, 8:16, :], in_=wqp[:, 8:16, :])
            nc.scalar.dma_start(out=wqT_sb[:, 16:24, :], in_=wqp[:, 16:24, :])
            nc.scalar.dma_start(out=wqT_sb[:, 24:32, :], in_=wqp[:, 24:32, :])
            nc.scalar.dma_start(out=wqT_sb[:, 0:8, :], in_=wqp[:, 0:8, :])
            nc.scalar.dma_start(out=cos_sb[:, :], in_=cosk[:, :])
            nc.scalar.dma_start(out=sin_sb[:, :], in_=sink[:, :])
            nc.scalar.dma_start(out=v_sb[:, 0 : SP // 128, :], in_=pvp[:, :, :])
            nc.scalar.dma_start(out=kT_sb[:, 0:SP], in_=pkT[:, :])

            # ---- pass A: chunk-outer accumulation of K, V, Q0, Q1 (8 banks).
            # The first QDEFER chunks contribute only K/V (light, 2.1us/chunk)
            # so the PE never catches the still-warming DMA stream; their
            # Q0/Q1 contributions run as a mini-sweep at the end of the pass
            # (accumulation order within a bank is free).
            QDEFER = 8
            kps = ph1_ps.tile([128, 2, 512], F32, tag="pa0")
            vps = ph1_ps.tile([128, 2, 512], F32, tag="pa1")
            q0ps = ph1_ps.tile([128, 2, 512], F32, tag="pa2")
            q1ps = ph1_ps.tile([128, 2, 512], F32, tag="pa3")

            def q01_mms(c, st_flags):
                for g in range(2):
                    nc.tensor.matmul(
                        q0ps[:, g, :],
                        lhsT=wqT_sb[:, c, 0:128],
                        rhs=xT_sb[:, c, ts(g, 512)],
                        **st_flags,
                    )
                    nc.tensor.matmul(
                        q1ps[:, g, :],
                        lhsT=wqT_sb[:, c, ds(128, 128)],
                        rhs=xT_sb[:, c, ts(g, 512)],
                        **st_flags,
                    )

            for c in range(HCH):
                st_flags = dict(start=(c == 0), stop=(c == HCH - 1))
                for g in range(2):
                    nc.tensor.matmul(
                        kps[:, g, :],
                        lhsT=wkT_sb[:, c, :],
                        rhs=xT_sb[:, c, ts(g, 512)],
                        **st_flags,
                    )
                    nc.tensor.matmul(
                        vps[:, g, :],
                        lhsT=wvT_sb[:, c, :],
                        rhs=xT_sb[:, c, ts(g, 512)],
                        **st_flags,
                    )
                if c >= QDEFER:
                    q01_mms(c, dict(start=(c == QDEFER), stop=False))
            for c in range(QDEFER):
                q01_mms(c, dict(start=False, stop=(c == QDEFER - 1)))

            # V psum -> bf16 staging (scalar; fast consumer frees pa1)
            vt_sb = proj.tile([128, 2, 512], BF16)
            nc.scalar.activation(
                vt_sb[:, :, :], vps[:, :, :], mybir.ActivationFunctionType.Copy
            )
            # K + Q0 + Q1 ropes on DVE (overlap the transposes / pass B2)
            for g in range(2):
                _rope_write(
                    nc, rope_tmp, kT_sb[:, ds(SP + g * 512, 512)], kps[:, g, :],
                    cos_sb, sin_sb, SP + g * 512, 512,
                )
            for g in range(2):
                _rope_write(
                    nc, rope_tmp, qT[0][:, ts(g, 512)], q0ps[:, g, :],
                    cos_sb, sin_sb, SP + g * 512, 512,
                )
            for g in range(2):
                _rope_write(
                    nc, rope_tmp, qT[1][:, ts(g, 512)], q1ps[:, g, :],
                    cos_sb, sin_sb, SP + g * 512, 512,
                )
            # V transposes into v_sb chunks [SP/128 ..): reuse pa1's banks
            # ([128,128] bf16 tiles in the slot vt's copy just freed)
            for k in range(8):
                ps2 = ph1_ps.tile([128, 128], BF16, tag="pa1", name="trps")
                nc.tensor.transpose(
                    ps2[:, :], vt_sb[:, k // 4, ts(k % 4, 128)], ident[:, :]
                )
                nc.scalar.copy(v_sb[:, SP // 128 + k, :], ps2[:, :])

            # ---- passes B2/B3: Q2 (reuses K's banks), Q3
            def q_pass(j, tag):
                qps = ph1_ps.tile([128, 2, 512], F32, tag=tag, name=f"q{j}ps")
                for c in range(HCH):
                    st_flags = dict(start=(c == 0), stop=(c == HCH - 1))
                    for g in range(2):
                        nc.tensor.matmul(
                            qps[:, g, :],
                            lhsT=wqT_sb[:, c, ts(j, 128)],
                            rhs=xT_sb[:, c, ts(g, 512)],
                            **st_flags,
                        )
                return qps

            q2ps = q_pass(2, "pa0")
            for g in range(2):
                _rope_write(
                    nc, rope_tmp, qT[2][:, ts(g, 512)], q2ps[:, g, :],
                    cos_sb, sin_sb, SP + g * 512, 512,
                )
            # B3 runs g-outer so the g0 bank's accumulation finishes mid-pass;
            # its PSUM-reading rope muls (stage 1) then overlap the g1 loop,
            # freeing B3's banks (aliased by the attention pools) early.
            q3ps = ph1_ps.tile([128, 2, 512], F32, tag="pa2", name="q3ps")
            q3t, q3u = [], []
            for g in range(2):
                for c in range(HCH):
                    nc.tensor.matmul(
                        q3ps[:, g, :],
                        lhsT=wqT_sb[:, c, ts(3, 128)],
                        rhs=xT_sb[:, c, ts(g, 512)],
                        start=(c == 0),
                        stop=(c == HCH - 1),
                    )
                cs = cos_sb[:, ds(SP + g * 512, 512)]
                sn = sin_sb[:, ds(SP + g * 512, 512)]
                t = rope_tmp.tile([128, 512], F32, tag="rope_t")
                u = rope_tmp.tile([128, 512], F32, tag="rope_u")
                nc.vector.tensor_mul(t[0:64, :], q3ps[64:128, g, :], sn[0:64, :])
                nc.vector.tensor_mul(t[64:128, :], q3ps[0:64, g, :], sn[64:128, :])
                nc.vector.tensor_mul(u[:, :], q3ps[:, g, :], cs)
                q3t.append(t)
                q3u.append(u)
            for g in range(2):
                dst = qT[3][:, ts(g, 512)]
                nc.vector.tensor_sub(dst[0:64, :], q3u[g][0:64, :], q3t[g][0:64, :])
                nc.vector.tensor_add(
                    dst[64:128, :], q3u[g][64:128, :], q3t[g][64:128, :]
                )

        # ============ Phase 2+3 interleaved: attention, o_proj, RS ==========
        # separate pools: scores (2 pairs in flight), o_proj accumulators
        # (double-buffered), and single-buffered sums/att (their consumers
        # drain within the next head's lead-in) — 4+2+1+1 = 8 banks
        st_ps = ctx.enter_context(tc.tile_pool(name="st_ps", bufs=2, space="PSUM"))
        ops_ps = ctx.enter_context(tc.tile_pool(name="ops_ps", bufs=2, space="PSUM"))
        sums_ps = ctx.enter_context(
            tc.tile_pool(name="sums_ps", bufs=1, space="PSUM")
        )
        at_ps = ctx.enter_context(tc.tile_pool(name="at_ps", bufs=1, space="PSUM"))

        pt_pool = ctx.enter_context(tc.tile_pool(name="pt", bufs=6))
        pa_pool = ctx.enter_context(tc.tile_pool(name="pa", bufs=4))
        rc_pool = ctx.enter_context(tc.tile_pool(name="rc", bufs=2))
        wo_pool = ctx.enter_context(tc.tile_pool(name="wo", bufs=1))
        # deep o_proj staging: a full RS chunk (2MB) fits in SBUF so the PE
        # never stalls on part-chunk DMA writes slowed by a concurrent RS
        ob_pool = ctx.enter_context(tc.tile_pool(name="ob", bufs=16))
        wo_sb = wo_pool.tile([128, HPC, H], BF16)
        nc.scalar.dma_start(out=wo_sb[:, :, :], in_=wop[:, :, :])



        def attention_cols(off, w):
            for h in range(HPC):
                sums = sums_ps.tile([128, 512], F32, tag="sums", name=f"s{h}{off}")
                att = at_ps.tile([128, 512], F32, tag="att", name=f"a{h}{off}")
                for cc in range(KVCH // 2):
                    st = st_ps.tile([128, 2, 512], F32, tag="st", name="st")
                    pt = pt_pool.tile([128, 2, 512], BF16, name="pt")
                    for j in range(2):
                        nc.tensor.matmul(
                            st[:, j, 0:w],
                            lhsT=kT_sb[:, ts(2 * cc + j, 128)],
                            rhs=qT[h][:, ds(off, w)],
                            start=True,
                            stop=True,
                        )
                    nc.scalar.activation(
                        pt[:, :, 0:w], st[:, :, 0:w], EXP, scale=INV_SQRT_D
                    )
                    # softmax denominator: DVE pre-sums the chunk pair (bf16),
                    # halving the PE's ones-matmul work
                    padd = pa_pool.tile([128, 512], BF16, name="padd")
                    nc.vector.tensor_add(
                        padd[:, 0:w], pt[:, 0, 0:w], pt[:, 1, 0:w]
                    )
                    nc.tensor.matmul(
                        sums[:, 0:w],
                        lhsT=ones_sb[:, :],
                        rhs=padd[:, 0:w],
                        start=(cc == 0),
                        stop=(cc == KVCH // 2 - 1),
                    )
                    for j in range(2):
                        c = 2 * cc + j
                        nc.tensor.matmul(
                            att[:, 0:w],
                            lhsT=v_sb[:, c, :],
                            rhs=pt[:, j, 0:w],
                            start=(c == 0),
                            stop=(c == KVCH - 1),
                        )
                recip = rc_pool.tile([128, 512], F32, name="recip")
                nc.vector.reciprocal_approx_fast(recip[:, 0:w], sums[:, 0:w])
                nc.vector.tensor_mul(
                    attnT[h][:, ds(off, w)], att[:, 0:w], recip[:, 0:w]
                )

        def oproj_chunk(k):
            for ii in range(CHUNK_ROWS[k] // 128):
                i = CHUNK_OFF[k] // 128 + ii
                for n in range(H // 512):
                    ps = ops_ps.tile([128, 512], F32, tag="ops", name="ops")
                    ob = ob_pool.tile([128, 512], BF16, name="ob")
                    for j in range(HPC):
                        nc.tensor.matmul(
                            ps[:, :],
                            lhsT=attnT[j][:, ts(i, 128)],
                            rhs=wo_sb[:, j, ts(n, 512)],
                            start=(j == 0),
                            stop=(j == HPC - 1),
                        )
                    nc.vector.tensor_copy(ob[:, :], ps[:, :])
                    # last chunk's writes go on the scalar queue (idle by
                    # then) so they spread across descriptor queues while
                    # racing the previous chunk's ReduceScatter
                    dmaq = nc.scalar if k == NCHUNK - 1 else nc.sync
                    dmaq.dma_start(
                        out=part_chunks[k][ts(ii, 128), ts(n, 512)],
                        in_=ob[:, :],
                    )
            nc.gpsimd.collective_compute(
                "ReduceScatter",
                mybir.AluOpType.add,
                ins=[part_chunks[k][:, :].opt()],
                outs=[rs_chunks[k][:, :].opt()],
                replica_groups=[list(range(NCORES))],
            )
            # gpsimd queue: an RS-gated trigger here can't block the sync
            # queue's part writes or the scalar queue's attention exps
            nc.gpsimd.dma_start(
                out=out_ext[ds(CHUNK_OFF[k] // NCORES, CHUNK_ROWS[k] // NCORES), :],
                in_=rs_chunks[k][:, :],
            )

        # attention in four 256-wide passes, each immediately followed by its
        # o_proj chunk + ReduceScatter: every RS overlaps the next group's
        # compute, and the collective stream starts ~50us earlier than with
        # 512-wide halves.
        attention_cols(0, 256)
        oproj_chunk(0)
        attention_cols(256, 256)
        oproj_chunk(1)
        attention_cols(512, 256)
        oproj_chunk(2)
        attention_cols(768, 256)
        oproj_chunk(3)

    nc.finalize()
    return nc


def _get_nc():
    if "nc" not in _NC_CACHE:
        _NC_CACHE["nc"] = _build_nc()
    return _NC_CACHE["nc"]


def _rope_tables():
    inv_freq = 1.0 / (ROPE_THETA ** (np.arange(0, D, 2, dtype=np.float32) / D))
    pos = np.arange(KV, dtype=np.float32)
    freqs = pos[:, None] * inv_freq[None, :]  # [KV, D/2]
    emb = np.concatenate([freqs, freqs], axis=-1)  # [KV, D]
    return np.cos(emb), np.sin(emb)  # [KV, D]


def _host_rope(x, cos, sin):
    # x: [S, D]; cos/sin: [S, D]
    x1, x2 = x[:, : D // 2], x[:, D // 2 :]
    rot = np.concatenate([-x2, x1], axis=-1)
    return x * cos + rot * sin


def _pack(mat_t, inner):
    """[n*128, inner] -> [128, n, inner]: SBUF layout, partition dim first."""
    n = mat_t.shape[0] // 128
    return np.ascontiguousarray(mat_t.reshape(n, 128, inner).transpose(1, 0, 2))


def kernel(hidden_states, past_k, past_v, Wq, Wk, Wv, Wo, trace=False):
    global LAST_RESULT
    bf = ml_dtypes.bfloat16
    x = np.asarray(hidden_states, dtype=np.float32)[0]  # [SQ, H]
    xTp = _pack(np.ascontiguousarray(x.T), SQ).astype(bf)
    cos, sin = _rope_tables()  # [KV, D] f32
    cosT = np.ascontiguousarray(cos.T).astype(bf)
    sinT = np.ascontiguousarray(sin.T).astype(bf)

    in_maps = []
    for m in range(NCORES):
        qr = slice(m * DQ, (m + 1) * DQ)
        kr = slice(m * D, (m + 1) * D)
        in_maps.append(
            {
                "xTp": xTp,
                "wqp": _pack(np.asarray(Wq)[qr].T, DQ).astype(bf),
                "wkp": _pack(np.asarray(Wk)[kr].T, D).astype(bf),
                "wvp": _pack(np.asarray(Wv)[kr].T, D).astype(bf),
                "wop": _pack(np.asarray(Wo)[:, qr].T, H).astype(bf),
                "pkT": np.ascontiguousarray(
                    _host_rope(
                        np.asarray(past_k, dtype=np.float32)[0, m], cos[:SP], sin[:SP]
                    ).T
                ).astype(bf),
                "pvp": _pack(np.asarray(past_v)[0, m], D).astype(bf),
                "cosk": cosT,
                "sink": sinT,
            }
        )

    nc = _get_nc()
    res = run_bass_kernel_spmd(
        nc, in_maps, core_ids=list(range(NCORES)), trace=trace
    )
    LAST_RESULT = res
    # Each core's "out" holds NCHUNK blocks of CHUNK_ROWS[k]/8 rows; block k
    # of core m is global rows CHUNK_OFF[k] + rsh_k*[m, m+1).
    out = np.empty((SQ, H), dtype=np.float32)
    for m in range(NCORES):
        shard = np.asarray(res.results[m]["out"], dtype=np.float32)
        for k in range(NCHUNK):
            rsh = CHUNK_ROWS[k] // NCORES
            soff = CHUNK_OFF[k] // NCORES
            out[CHUNK_OFF[k] + rsh * m : CHUNK_OFF[k] + rsh * (m + 1)] = shard[
                soff : soff + rsh
            ]
    return out.reshape(B, SQ, H)


# revision 65
# speedup vs baseline: 1.1084x; 1.0058x over previous
"""Tensor-parallel GQA attention block (AtlasAttentionWrapper) on 8 TRN2 cores.

Sharding: TP over heads. Core m owns query heads [4m..4m+3] (Wq rows
m*512:(m+1)*512), KV head m (Wk/Wv rows m*128:(m+1)*128, past_k/past_v head m)
and Wo columns m*512:(m+1)*512. Each core computes a full [1024, 4096] o_proj
partial; chunked ReduceScatters ([512, 256, 256] rows, pipelined under the
remaining attention/o_proj compute) leave each core 1/8 of the rows of each
chunk; the host reassembles.

All device inputs are host-packed into SBUF layout [128, chunks, inner] so
every DMA is a large transfer with long contiguous per-partition lines.

Phase 1 runs chunk-outer with persistent PSUM accumulators so the PE starts
as soon as the first weight/xT pieces land:
  pass A  accumulates K, V, Q0 (6 banks) over all 32 contraction chunks,
  passes B1..B3 accumulate Q1..Q3 (2 banks each), reusing banks whose
  consumers (V copy, K rope, Q0 rope) have already drained.
Each rope is emitted right after its producing pass so DVE overlaps the next
pass's matmuls; per-head qT tiles keep attention from waiting on later ropes.
Attention (scores built transposed, exp on scalar with 2-chunk batching,
softmax denominator via ones-matmul) and o_proj + chunked ReduceScatter are
interleaved: attn(g0), oproj k0/k1 + RS, attn(g1), oproj k2/k3 + RS.
"""

import sys

if "/opt/trn_rl_repo" not in sys.path:
    sys.path.insert(0, "/opt/trn_rl_repo")

from contextlib import ExitStack

import ml_dtypes
import numpy as np

import concourse.bass as bass
import concourse.tile as tile
from concourse import bacc, mybir
from concourse.bass import ds, ts
from concourse.bass_utils import run_bass_kernel_spmd
from concourse.masks import make_identity

NCORES = 8
B, SQ, H = 1, 1024, 4096
NH, NKV, D = 32, 8, 128
SP = 1024
KV = SP + SQ  # 2048
HPC = NH // NCORES  # 4 query heads per core
DQ = HPC * D  # 512
SH = SQ // NCORES  # 128 output rows per core after ReduceScatter
ROPE_THETA = 10000.0
INV_SQRT_D = 1.0 / float(np.sqrt(D))

BF16 = mybir.dt.bfloat16
F32 = mybir.dt.float32
HCH = H // 128  # 32 contraction chunks
KVCH = KV // 128  # 16 kv chunks
# ReduceScatter chunk sizes (rows), matching the attention column groups:
# early 2MB chunks start the (HBM-bound, continuously-busy) collective
# stream as soon as possible; the final narrow group + 1MB chunk pulls the
# last, fully-exposed RS forward.
CHUNK_ROWS = [256, 256, 256, 256]
CHUNK_OFF = [0, 256, 512, 768]
NCHUNK = len(CHUNK_ROWS)
EXP = mybir.ActivationFunctionType.Exp

LAST_RESULT = None
_NC_CACHE = {}


def _rope_write(nc, tmp_pool, dst, src, cos_sb, sin_sb, pos, width):
    """dst[d, s] = rope(src)[d, s] for s in [pos, pos+width) absolute positions.

    src: AP [128, width] (PSUM f32 or SBUF bf16), dst: SBUF bf16 AP.
    rope: out[d<64] = x[d]*cos[d] - x[d+64]*sin[d]
          out[d>=64] = x[d]*cos[d] + x[d-64]*sin[d]
    """
    cs = cos_sb[:, ds(pos, width)]
    sn = sin_sb[:, ds(pos, width)]
    t = tmp_pool.tile([128, width], F32, tag="rope_t")
    u = tmp_pool.tile([128, width], F32, tag="rope_u")
    nc.vector.tensor_mul(t[0:64, :], src[64:128, :], sn[0:64, :])
    nc.vector.tensor_mul(t[64:128, :], src[0:64, :], sn[64:128, :])
    nc.vector.tensor_mul(u[:, :], src[:, :], cs)
    nc.vector.tensor_sub(dst[0:64, :], u[0:64, :], t[0:64, :])
    nc.vector.tensor_add(dst[64:128, :], u[64:128, :], t[64:128, :])


def _build_nc():
    nc = bacc.Bacc(None, target_bir_lowering=False, debug=False)

    xTp = nc.declare_dram_parameter("xTp", [128, HCH, SQ], BF16, False)
    wqp = nc.declare_dram_parameter("wqp", [128, HCH, DQ], BF16, False)
    wkp = nc.declare_dram_parameter("wkp", [128, HCH, D], BF16, False)
    wvp = nc.declare_dram_parameter("wvp", [128, HCH, D], BF16, False)
    wop = nc.declare_dram_parameter("wop", [128, HPC, H], BF16, False)
    pkT = nc.declare_dram_parameter("pkT", [D, SP], BF16, False)
    pvp = nc.declare_dram_parameter("pvp", [128, SP // 128, D], BF16, False)
    cosk = nc.declare_dram_parameter("cosk", [D, KV], BF16, False)
    sink = nc.declare_dram_parameter("sink", [D, KV], BF16, False)
    out_ext = nc.declare_dram_parameter("out", [SH, H], BF16, True)

    with tile.TileContext(nc) as tc, ExitStack() as ctx:
        # ---- persistent SBUF residents (live across all phases)
        const = ctx.enter_context(tc.tile_pool(name="const", bufs=1))
        kT_sb = const.tile([128, KV], BF16)  # roped K^T  [d, kv]
        v_sb = const.tile([128, KVCH, D], BF16)  # V chunks [kv%128, chunk, d]
        # per-head roped Q^T / attn^T tiles (separate tiles keep readers from
        # waiting on later heads' writes)
        qT = [const.tile([128, SQ], BF16, name=f"qT{j}") for j in range(HPC)]
        attnT = [const.tile([128, SQ], BF16, name=f"attnT{j}") for j in range(HPC)]
        cos_sb = const.tile([128, KV], BF16)
        sin_sb = const.tile([128, KV], BF16)
        ident = const.tile([128, 128], BF16)
        ones_sb = const.tile([128, 128], BF16)

        make_identity(nc, ident[:, :])
        nc.vector.memset(ones_sb[:, :], 1.0)

        rope_tmp = ctx.enter_context(tc.tile_pool(name="rope_tmp", bufs=2))
        dram = ctx.enter_context(tc.tile_pool(name="dram", bufs=1, space="DRAM"))
        part_chunks = []
        rs_chunks = []
        for k in range(NCHUNK):
            part_chunks.append(
                dram.tile([CHUNK_ROWS[k], H], BF16, tag=f"part{k}", name=f"part{k}")
            )
            rs_chunks.append(
                dram.tile(
                    [CHUNK_ROWS[k] // NCORES, H], BF16, tag=f"rs{k}", name=f"rs{k}"
                )
            )

        # ================= Phase 1: projections + rope ==================
        with tc.tile_pool(name="proj", bufs=1) as proj, tc.tile_pool(
            name="ph1_ps", bufs=1, space="PSUM"
        ) as ph1_ps:
            xT_sb = proj.tile([128, HCH, SQ], BF16)
            wqT_sb = proj.tile([128, HCH, DQ], BF16)
            wkT_sb = proj.tile([128, HCH, D], BF16)
            wvT_sb = proj.tile([128, HCH, D], BF16)

            # Weight/xT stream: small first pieces so the chunk-outer pass A
            # starts within a few us; later pieces sized to stay ahead of PE.
            nc.sync.dma_start(out=wkT_sb[:, 0:2, :], in_=wkp[:, 0:2, :])
            nc.sync.dma_start(out=xT_sb[:, 0:1, :], in_=xTp[:, 0:1, :])
            nc.sync.dma_start(out=wvT_sb[:, 0:2, :], in_=wvp[:, 0:2, :])
            nc.sync.dma_start(out=xT_sb[:, 1:2, :], in_=xTp[:, 1:2, :])
            nc.sync.dma_start(out=wkT_sb[:, 2:8, :], in_=wkp[:, 2:8, :])
            nc.sync.dma_start(out=wvT_sb[:, 2:8, :], in_=wvp[:, 2:8, :])
            nc.sync.dma_start(out=xT_sb[:, 2:4, :], in_=xTp[:, 2:4, :])
            nc.sync.dma_start(out=xT_sb[:, 4:6, :], in_=xTp[:, 4:6, :])
            nc.sync.dma_start(out=wkT_sb[:, 8:16, :], in_=wkp[:, 8:16, :])
            nc.sync.dma_start(out=wvT_sb[:, 8:16, :], in_=wvp[:, 8:16, :])
            nc.sync.dma_start(out=xT_sb[:, 6:8, :], in_=xTp[:, 6:8, :])
            nc.sync.dma_start(out=xT_sb[:, 8:12, :], in_=xTp[:, 8:12, :])
            nc.sync.dma_start(out=xT_sb[:, 12:16, :], in_=xTp[:, 12:16, :])
            nc.sync.dma_start(out=wkT_sb[:, 16:32, :], in_=wkp[:, 16:32, :])
            nc.sync.dma_start(out=wvT_sb[:, 16:32, :], in_=wvp[:, 16:32, :])
            nc.sync.dma_start(out=xT_sb[:, 16:20, :], in_=xTp[:, 16:20, :])
            nc.sync.dma_start(out=xT_sb[:, 20:24, :], in_=xTp[:, 20:24, :])
            nc.sync.dma_start(out=xT_sb[:, 24:32, :], in_=xTp[:, 24:32, :])
            # scalar queue: Q weights in consumption order (chunks >= QDEFER
            # are needed first; chunks 0:8 only at the end-of-A mini-sweep),
            # then the small rope/past tensors (first needed at the K rope).
            nc.scalar.dma_start(out=wqT_sb[:, 8:16, :], in_=wqp[:, 8:16, :])
            nc.scalar.dma_start(out=wqT_sb[:, 16:24, :], in_=wqp[:, 16:24, :])
            nc.scalar.dma_start(out=wqT_sb[:, 24:32, :], in_=wqp[:, 24:32, :])
            nc.scalar.dma_start(out=wqT_sb[:, 0:8, :], in_=wqp[:, 0:8, :])
            nc.scalar.dma_start(out=cos_sb[:, :], in_=cosk[:, :])
            nc.scalar.dma_start(out=sin_sb[:, :], in_=sink[:, :])
            nc.scalar.dma_start(out=v_sb[:, 0 : SP // 128, :], in_=pvp[:, :, :])
            nc.scalar.dma_start(out=kT_sb[:, 0:SP], in_=pkT[:, :])

            # ---- pass A: chunk-outer accumulation of K, V, Q0, Q1 (8 banks).
            # The first QDEFER chunks contribute only K/V (light, 2.1us/chunk)
            # so the PE never catches the still-warming DMA stream; their
            # Q0/Q1 contributions run as a mini-sweep at the end of the pass
            # (accumulation order within a bank is free).
            QDEFER = 8
            kps = ph1_ps.tile([128, 2, 512], F32, tag="pa0")
            vps = ph1_ps.tile([128, 2, 512], F32, tag="pa1")
            q0ps = ph1_ps.tile([128, 2, 512], F32, tag="pa2")
            q1ps = ph1_ps.tile([128, 2, 512], F32, tag="pa3")

            def q01_mms(c, st_flags):
                for g in range(2):
                    nc.tensor.matmul(
                        q0ps[:, g, :],
                        lhsT=wqT_sb[:, c, 0:128],
                        rhs=xT_sb[:, c, ts(g, 512)],
                        **st_flags,
                    )
                    nc.tensor.matmul(
                        q1ps[:, g, :],
                        lhsT=wqT_sb[:, c, ds(128, 128)],
                        rhs=xT_sb[:, c, ts(g, 512)],
                        **st_flags,
                    )

            for c in range(HCH):
                st_flags = dict(start=(c == 0), stop=(c == HCH - 1))
                for g in range(2):
                    nc.tensor.matmul(
                        kps[:, g, :],
                        lhsT=wkT_sb[:, c, :],
                        rhs=xT_sb[:, c, ts(g, 512)],
                        **st_flags,
                    )
                    nc.tensor.matmul(
                        vps[:, g, :],
                        lhsT=wvT_sb[:, c, :],
                        rhs=xT_sb[:, c, ts(g, 512)],
                        **st_flags,
                    )
                if c >= QDEFER:
                    q01_mms(c, dict(start=(c == QDEFER), stop=False))
            for c in range(QDEFER):
                q01_mms(c, dict(start=False, stop=(c == QDEFER - 1)))

            # V psum -> bf16 staging (scalar; fast consumer frees pa1)
            vt_sb = proj.tile([128, 2, 512], BF16)
            nc.scalar.activation(
                vt_sb[:, :, :], vps[:, :, :], mybir.ActivationFunctionType.Copy
            )
            # K + Q0 + Q1 ropes on DVE (overlap the transposes / pass B2)
            for g in range(2):
                _rope_write(
                    nc, rope_tmp, kT_sb[:, ds(SP + g * 512, 512)], kps[:, g, :],
                    cos_sb, sin_sb, SP + g * 512, 512,
                )
            for g in range(2):
                _rope_write(
                    nc, rope_tmp, qT[0][:, ts(g, 512)], q0ps[:, g, :],
                    cos_sb, sin_sb, SP + g * 512, 512,
                )
            for g in range(2):
                _rope_write(
                    nc, rope_tmp, qT[1][:, ts(g, 512)], q1ps[:, g, :],
                    cos_sb, sin_sb, SP + g * 512, 512,
                )
            # V transposes into v_sb chunks [SP/128 ..): reuse pa1's banks
            # ([128,128] bf16 tiles in the slot vt's copy just freed)
            for k in range(8):
                ps2 = ph1_ps.tile([128, 128], BF16, tag="pa1", name="trps")
                nc.tensor.transpose(
                    ps2[:, :], vt_sb[:, k // 4, ts(k % 4, 128)], ident[:, :]
                )
                nc.scalar.copy(v_sb[:, SP // 128 + k, :], ps2[:, :])

            # ---- passes B2/B3: Q2 (reuses K's banks), Q3
            def q_pass(j, tag):
                qps = ph1_ps.tile([128, 2, 512], F32, tag=tag, name=f"q{j}ps")
                for c in range(HCH):
                    st_flags = dict(start=(c == 0), stop=(c == HCH - 1))
                    for g in range(2):
                        nc.tensor.matmul(
                            qps[:, g, :],
                            lhsT=wqT_sb[:, c, ts(j, 128)],
                            rhs=xT_sb[:, c, ts(g, 512)],
                            **st_flags,
                        )
                return qps

            q2ps = q_pass(2, "pa0")
            for g in range(2):
                _rope_write(
                    nc, rope_tmp, qT[2][:, ts(g, 512)], q2ps[:, g, :],
                    cos_sb, sin_sb, SP + g * 512, 512,
                )
            # B3 runs g-outer so the g0 bank's accumulation finishes mid-pass;
            # its PSUM-reading rope muls (stage 1) then overlap the g1 loop,
            # freeing B3's banks (aliased by the attention pools) early.
            q3ps = ph1_ps.tile([128, 2, 512], F32, tag="pa2", name="q3ps")
            q3t, q3u = [], []
            for g in range(2):
                for c in range(HCH):
                    nc.tensor.matmul(
                        q3ps[:, g, :],
                        lhsT=wqT_sb[:, c, ts(3, 128)],
                        rhs=xT_sb[:, c, ts(g, 512)],
                        start=(c == 0),
                        stop=(c == HCH - 1),
                    )
                cs = cos_sb[:, ds(SP + g * 512, 512)]
                sn = sin_sb[:, ds(SP + g * 512, 512)]
                t = rope_tmp.tile([128, 512], F32, tag="rope_t")
                u = rope_tmp.tile([128, 512], F32, tag="rope_u")
                nc.vector.tensor_mul(t[0:64, :], q3ps[64:128, g, :], sn[0:64, :])
                nc.vector.tensor_mul(t[64:128, :], q3ps[0:64, g, :], sn[64:128, :])
                nc.vector.tensor_mul(u[:, :], q3ps[:, g, :], cs)
                q3t.append(t)
                q3u.append(u)
            for g in range(2):
                dst = qT[3][:, ts(g, 512)]
                nc.vector.tensor_sub(dst[0:64, :], q3u[g][0:64, :], q3t[g][0:64, :])
                nc.vector.tensor_add(
                    dst[64:128, :], q3u[g][64:128, :], q3t[g][64:128, :]
                )

        # ============ Phase 2+3 interleaved: attention, o_proj, RS ==========
        # separate pools: scores (2 pairs in flight), o_proj accumulators
        # (double-buffered), and single-buffered sums/att (their consumers
        # drain within the next head's lead-in) — 4+2+1+1 = 8 banks
        st_ps = ctx.enter_context(tc.tile_pool(name="st_ps", bufs=2, space="PSUM"))
        ops_ps = ctx.enter_context(tc.tile_pool(name="ops_ps", bufs=2, space="PSUM"))
        sums_ps = ctx.enter_context(
            tc.tile_pool(name="sums_ps", bufs=1, space="PSUM")
        )
        at_ps = ctx.enter_context(tc.tile_pool(name="at_ps", bufs=1, space="PSUM"))

        pt_pool = ctx.enter_context(tc.tile_pool(name="pt", bufs=6))
        pa_pool = ctx.enter_context(tc.tile_pool(name="pa", bufs=4))
        rc_pool = ctx.enter_context(tc.tile_pool(name="rc", bufs=2))
        wo_pool = ctx.enter_context(tc.tile_pool(name="wo", bufs=1))
        # deep o_proj staging: a full RS chunk (2MB) fits in SBUF so the PE
        # never stalls on part-chunk DMA writes slowed by a concurrent RS
        ob_pool = ctx.enter_context(tc.tile_pool(name="ob", bufs=16))
        wo_sb = wo_pool.tile([128, HPC, H], BF16)
        nc.scalar.dma_start(out=wo_sb[:, :, :], in_=wop[:, :, :])



        def attention_cols(off, w):
            for h in range(HPC):
                sums = sums_ps.tile([128, 512], F32, tag="sums", name=f"s{h}{off}")
                att = at_ps.tile([128, 512], F32, tag="att", name=f"a{h}{off}")
                for cc in range(KVCH // 2):
                    st = st_ps.tile([128, 2, 512], F32, tag="st", name="st")
                    pt = pt_pool.tile([128, 2, 512], BF16, name="pt")
                    for j in range(2):
                        nc.tensor.matmul(
                            st[:, j, 0:w],
                            lhsT=kT_sb[:, ts(2 * cc + j, 128)],
                            rhs=qT[h][:, ds(off, w)],
                            start=True,
                            stop=True,
                        )
                    nc.scalar.activation(
                        pt[:, :, 0:w], st[:, :, 0:w], EXP, scale=INV_SQRT_D
                    )
                    # softmax denominator: DVE pre-sums the chunk pair (bf16),
                    # halving the PE's ones-matmul work
                    padd = pa_pool.tile([128, 512], BF16, name="padd")
                    nc.vector.tensor_add(
                        padd[:, 0:w], pt[:, 0, 0:w], pt[:, 1, 0:w]
                    )
                    nc.tensor.matmul(
                        sums[:, 0:w],
                        lhsT=ones_sb[:, :],
                        rhs=padd[:, 0:w],
                        start=(cc == 0),
                        stop=(cc == KVCH // 2 - 1),
                    )
                    for j in range(2):
                        c = 2 * cc + j
                        nc.tensor.matmul(
                            att[:, 0:w],
                            lhsT=v_sb[:, c, :],
                            rhs=pt[:, j, 0:w],
                            start=(c == 0),
                            stop=(c == KVCH - 1),
                        )
                recip = rc_pool.tile([128, 512], F32, name="recip")
                nc.vector.reciprocal_approx_fast(recip[:, 0:w], sums[:, 0:w])
                nc.vector.tensor_mul(
                    attnT[h][:, ds(off, w)], att[:, 0:w], recip[:, 0:w]
                )

        def oproj_chunk(k):
            for ii in range(CHUNK_ROWS[k] // 128):
                i = CHUNK_OFF[k] // 128 + ii
                for n in range(H // 512):
                    ps = ops_ps.tile([128, 512], F32, tag="ops", name="ops")
                    ob = ob_pool.tile([128, 512], BF16, name="ob")
                    for j in range(HPC):
                        nc.tensor.matmul(
                            ps[:, :],
                            lhsT=attnT[j][:, ts(i, 128)],
                            rhs=wo_sb[:, j, ts(n, 512)],
                            start=(j == 0),
                            stop=(j == HPC - 1),
                        )
                    nc.vector.tensor_copy(ob[:, :], ps[:, :])
                    # last chunk: alternate descriptor rings (both idle by
                    # then) — the DMA engines serve active rings round-robin,
                    # so two rings roughly double the writes' share against
                    # the concurrent ReduceScatter's reserved rings
                    if k == NCHUNK - 1:
                        dmaq = nc.scalar if n % 2 else nc.sync
                    else:
                        dmaq = nc.sync
                    dmaq.dma_start(
                        out=part_chunks[k][ts(ii, 128), ts(n, 512)],
                        in_=ob[:, :],
                    )
            nc.gpsimd.collective_compute(
                "ReduceScatter",
                mybir.AluOpType.add,
                ins=[part_chunks[k][:, :].opt()],
                outs=[rs_chunks[k][:, :].opt()],
                replica_groups=[list(range(NCORES))],
            )
            # gpsimd queue: an RS-gated trigger here can't block the sync
            # queue's part writes or the scalar queue's attention exps
            nc.gpsimd.dma_start(
                out=out_ext[ds(CHUNK_OFF[k] // NCORES, CHUNK_ROWS[k] // NCORES), :],
                in_=rs_chunks[k][:, :],
            )

        # attention in four 256-wide passes, each immediately followed by its
        # o_proj chunk + ReduceScatter: every RS overlaps the next group's
        # compute, and the collective stream starts ~50us earlier than with
        # 512-wide halves.
        attention_cols(0, 256)
        oproj_chunk(0)
        attention_cols(256, 256)
        oproj_chunk(1)
        attention_cols(512, 256)
        oproj_chunk(2)
        attention_cols(768, 256)
        oproj_chunk(3)

    nc.finalize()
    return nc


def _get_nc():
    if "nc" not in _NC_CACHE:
        _NC_CACHE["nc"] = _build_nc()
    return _NC_CACHE["nc"]


def _rope_tables():
    inv_freq = 1.0 / (ROPE_THETA ** (np.arange(0, D, 2, dtype=np.float32) / D))
    pos = np.arange(KV, dtype=np.float32)
    freqs = pos[:, None] * inv_freq[None, :]  # [KV, D/2]
    emb = np.concatenate([freqs, freqs], axis=-1)  # [KV, D]
    return np.cos(emb), np.sin(emb)  # [KV, D]


def _host_rope(x, cos, sin):
    # x: [S, D]; cos/sin: [S, D]
    x1, x2 = x[:, : D // 2], x[:, D // 2 :]
    rot = np.concatenate([-x2, x1], axis=-1)
    return x * cos + rot * sin


def _pack(mat_t, inner):
    """[n*128, inner] -> [128, n, inner]: SBUF layout, partition dim first."""
    n = mat_t.shape[0] // 128
    return np.ascontiguousarray(mat_t.reshape(n, 128, inner).transpose(1, 0, 2))


def kernel(hidden_states, past_k, past_v, Wq, Wk, Wv, Wo, trace=False):
    global LAST_RESULT
    bf = ml_dtypes.bfloat16
    x = np.asarray(hidden_states, dtype=np.float32)[0]  # [SQ, H]
    xTp = _pack(np.ascontiguousarray(x.T), SQ).astype(bf)
    cos, sin = _rope_tables()  # [KV, D] f32
    cosT = np.ascontiguousarray(cos.T).astype(bf)
    sinT = np.ascontiguousarray(sin.T).astype(bf)

    in_maps = []
    for m in range(NCORES):
        qr = slice(m * DQ, (m + 1) * DQ)
        kr = slice(m * D, (m + 1) * D)
        in_maps.append(
            {
                "xTp": xTp,
                "wqp": _pack(np.asarray(Wq)[qr].T, DQ).astype(bf),
                "wkp": _pack(np.asarray(Wk)[kr].T, D).astype(bf),
                "wvp": _pack(np.asarray(Wv)[kr].T, D).astype(bf),
                "wop": _pack(np.asarray(Wo)[:, qr].T, H).astype(bf),
                "pkT": np.ascontiguousarray(
                    _host_rope(
                        np.asarray(past_k, dtype=np.float32)[0, m], cos[:SP], sin[:SP]
                    ).T
                ).astype(bf),
                "pvp": _pack(np.asarray(past_v)[0, m], D).astype(bf),
                "cosk": cosT,
                "sink": sinT,
            }
        )

    nc = _get_nc()
    res = run_bass_kernel_spmd(
        nc, in_maps, core_ids=list(range(NCORES)), trace=trace
    )
    LAST_RESULT = res
    # Each core's "out" holds NCHUNK blocks of CHUNK_ROWS[k]/8 rows; block k
    # of core m is global rows CHUNK_OFF[k] + rsh_k*[m, m+1).
    out = np.empty((SQ, H), dtype=np.float32)
    for m in range(NCORES):
        shard = np.asarray(res.results[m]["out"], dtype=np.float32)
        for k in range(NCHUNK):
            rsh = CHUNK_ROWS[k] // NCORES
            soff = CHUNK_OFF[k] // NCORES
            out[CHUNK_OFF[k] + rsh * m : CHUNK_OFF[k] + rsh * (m + 1)] = shard[
                soff : soff + rsh
            ]
    return out.reshape(B, SQ, H)


# revision 66
# speedup vs baseline: 1.1346x; 1.0236x over previous
"""Tensor-parallel GQA attention block (AtlasAttentionWrapper) on 8 TRN2 cores.

Sharding: TP over heads. Core m owns query heads [4m..4m+3] (Wq rows
m*512:(m+1)*512), KV head m (Wk/Wv rows m*128:(m+1)*128, past_k/past_v head m)
and Wo columns m*512:(m+1)*512. Each core computes a full [1024, 4096] o_proj
partial; chunked ReduceScatters ([512, 256, 256] rows, pipelined under the
remaining attention/o_proj compute) leave each core 1/8 of the rows of each
chunk; the host reassembles.

All device inputs are host-packed into SBUF layout [128, chunks, inner] so
every DMA is a large transfer with long contiguous per-partition lines.

Phase 1 runs chunk-outer with persistent PSUM accumulators so the PE starts
as soon as the first weight/xT pieces land:
  pass A  accumulates K, V, Q0 (6 banks) over all 32 contraction chunks,
  passes B1..B3 accumulate Q1..Q3 (2 banks each), reusing banks whose
  consumers (V copy, K rope, Q0 rope) have already drained.
Each rope is emitted right after its producing pass so DVE overlaps the next
pass's matmuls; per-head qT tiles keep attention from waiting on later ropes.
Attention (scores built transposed, exp on scalar with 2-chunk batching,
softmax denominator via ones-matmul) and o_proj + chunked ReduceScatter are
interleaved: attn(g0), oproj k0/k1 + RS, attn(g1), oproj k2/k3 + RS.
"""

import sys

if "/opt/trn_rl_repo" not in sys.path:
    sys.path.insert(0, "/opt/trn_rl_repo")

from contextlib import ExitStack

import ml_dtypes
import numpy as np

import concourse.bass as bass
import concourse.tile as tile
from concourse import bacc, mybir
from concourse.bass import ds, ts
from concourse.bass_utils import run_bass_kernel_spmd
from concourse.masks import make_identity

NCORES = 8
B, SQ, H = 1, 1024, 4096
NH, NKV, D = 32, 8, 128
SP = 1024
KV = SP + SQ  # 2048
HPC = NH // NCORES  # 4 query heads per core
DQ = HPC * D  # 512
SH = SQ // NCORES  # 128 output rows per core after ReduceScatter
ROPE_THETA = 10000.0
INV_SQRT_D = 1.0 / float(np.sqrt(D))

BF16 = mybir.dt.bfloat16
F32 = mybir.dt.float32
HCH = H // 128  # 32 contraction chunks
KVCH = KV // 128  # 16 kv chunks
# ReduceScatter chunk sizes (rows), matching the attention column groups:
# early 2MB chunks start the (HBM-bound, continuously-busy) collective
# stream as soon as possible; the final narrow group + 1MB chunk pulls the
# last, fully-exposed RS forward.
CHUNK_ROWS = [256, 256, 256, 256]
CHUNK_OFF = [0, 256, 512, 768]
NCHUNK = len(CHUNK_ROWS)
EXP = mybir.ActivationFunctionType.Exp

LAST_RESULT = None
_NC_CACHE = {}


def _rope_write(nc, tmp_pool, dst, src, cos_sb, sin_sb, pos, width):
    """dst[d, s] = rope(src)[d, s] for s in [pos, pos+width) absolute positions.

    src: AP [128, width] (PSUM f32 or SBUF bf16), dst: SBUF bf16 AP.
    rope: out[d<64] = x[d]*cos[d] - x[d+64]*sin[d]
          out[d>=64] = x[d]*cos[d] + x[d-64]*sin[d]
    """
    cs = cos_sb[:, ds(pos, width)]
    sn = sin_sb[:, ds(pos, width)]
    t = tmp_pool.tile([128, width], F32, tag="rope_t")
    u = tmp_pool.tile([128, width], F32, tag="rope_u")
    nc.vector.tensor_mul(t[0:64, :], src[64:128, :], sn[0:64, :])
    nc.vector.tensor_mul(t[64:128, :], src[0:64, :], sn[64:128, :])
    nc.vector.tensor_mul(u[:, :], src[:, :], cs)
    nc.vector.tensor_sub(dst[0:64, :], u[0:64, :], t[0:64, :])
    nc.vector.tensor_add(dst[64:128, :], u[64:128, :], t[64:128, :])


def _build_nc():
    nc = bacc.Bacc(None, target_bir_lowering=False, debug=False)

    xTp = nc.declare_dram_parameter("xTp", [128, HCH, SQ], BF16, False)
    wqp = nc.declare_dram_parameter("wqp", [128, HCH, DQ], BF16, False)
    wkp = nc.declare_dram_parameter("wkp", [128, HCH, D], BF16, False)
    wvp = nc.declare_dram_parameter("wvp", [128, HCH, D], BF16, False)
    wop = nc.declare_dram_parameter("wop", [128, HPC, H], BF16, False)
    pkT = nc.declare_dram_parameter("pkT", [D, SP], BF16, False)
    pvp = nc.declare_dram_parameter("pvp", [128, SP // 128, D], BF16, False)
    cosk = nc.declare_dram_parameter("cosk", [D, KV], BF16, False)
    sink = nc.declare_dram_parameter("sink", [D, KV], BF16, False)
    out_ext = nc.declare_dram_parameter("out", [SH, H], BF16, True)

    with tile.TileContext(nc) as tc, ExitStack() as ctx:
        # ---- persistent SBUF residents (live across all phases)
        const = ctx.enter_context(tc.tile_pool(name="const", bufs=1))
        kT_sb = const.tile([128, KV], BF16)  # roped K^T  [d, kv]
        v_sb = const.tile([128, KVCH, D], BF16)  # V chunks [kv%128, chunk, d]
        # per-head roped Q^T / attn^T tiles (separate tiles keep readers from
        # waiting on later heads' writes)
        qT = [const.tile([128, SQ], BF16, name=f"qT{j}") for j in range(HPC)]
        attnT = [const.tile([128, SQ], BF16, name=f"attnT{j}") for j in range(HPC)]
        cos_sb = const.tile([128, KV], BF16)
        sin_sb = const.tile([128, KV], BF16)
        ident = const.tile([128, 128], BF16)
        ones_sb = const.tile([128, 128], BF16)

        make_identity(nc, ident[:, :])
        nc.vector.memset(ones_sb[:, :], 1.0)

        rope_tmp = ctx.enter_context(tc.tile_pool(name="rope_tmp", bufs=2))
        dram = ctx.enter_context(tc.tile_pool(name="dram", bufs=1, space="DRAM"))
        part_chunks = []
        rs_chunks = []
        for k in range(NCHUNK):
            part_chunks.append(
                dram.tile([CHUNK_ROWS[k], H], BF16, tag=f"part{k}", name=f"part{k}")
            )
            rs_chunks.append(
                dram.tile(
                    [CHUNK_ROWS[k] // NCORES, H], BF16, tag=f"rs{k}", name=f"rs{k}"
                )
            )

        # ================= Phase 1: projections + rope ==================
        with tc.tile_pool(name="proj", bufs=1) as proj, tc.tile_pool(
            name="ph1_ps", bufs=1, space="PSUM"
        ) as ph1_ps:
            xT_sb = proj.tile([128, HCH, SQ], BF16)
            wqT_sb = proj.tile([128, HCH, DQ], BF16)
            wkT_sb = proj.tile([128, HCH, D], BF16)
            wvT_sb = proj.tile([128, HCH, D], BF16)

            # Weight/xT stream: small first pieces so the chunk-outer pass A
            # starts within a few us; later pieces sized to stay ahead of PE.
            nc.sync.dma_start(out=wkT_sb[:, 0:2, :], in_=wkp[:, 0:2, :])
            nc.sync.dma_start(out=xT_sb[:, 0:1, :], in_=xTp[:, 0:1, :])
            nc.sync.dma_start(out=wvT_sb[:, 0:2, :], in_=wvp[:, 0:2, :])
            nc.sync.dma_start(out=xT_sb[:, 1:2, :], in_=xTp[:, 1:2, :])
            nc.sync.dma_start(out=wkT_sb[:, 2:8, :], in_=wkp[:, 2:8, :])
            nc.sync.dma_start(out=wvT_sb[:, 2:8, :], in_=wvp[:, 2:8, :])
            nc.sync.dma_start(out=xT_sb[:, 2:4, :], in_=xTp[:, 2:4, :])
            nc.sync.dma_start(out=xT_sb[:, 4:6, :], in_=xTp[:, 4:6, :])
            nc.sync.dma_start(out=wkT_sb[:, 8:16, :], in_=wkp[:, 8:16, :])
            nc.sync.dma_start(out=wvT_sb[:, 8:16, :], in_=wvp[:, 8:16, :])
            nc.sync.dma_start(out=xT_sb[:, 6:8, :], in_=xTp[:, 6:8, :])
            nc.sync.dma_start(out=xT_sb[:, 8:12, :], in_=xTp[:, 8:12, :])
            nc.sync.dma_start(out=xT_sb[:, 12:16, :], in_=xTp[:, 12:16, :])
            nc.sync.dma_start(out=wkT_sb[:, 16:32, :], in_=wkp[:, 16:32, :])
            nc.sync.dma_start(out=wvT_sb[:, 16:32, :], in_=wvp[:, 16:32, :])
            nc.sync.dma_start(out=xT_sb[:, 16:20, :], in_=xTp[:, 16:20, :])
            nc.sync.dma_start(out=xT_sb[:, 20:24, :], in_=xTp[:, 20:24, :])
            nc.sync.dma_start(out=xT_sb[:, 24:32, :], in_=xTp[:, 24:32, :])
            # scalar queue: Q weights in consumption order (chunks >= QDEFER
            # are needed first; chunks 0:8 only at the end-of-A mini-sweep),
            # then the small rope/past tensors (first needed at the K rope).
            nc.scalar.dma_start(out=wqT_sb[:, 8:16, :], in_=wqp[:, 8:16, :])
            nc.scalar.dma_start(out=wqT_sb[:, 16:24, :], in_=wqp[:, 16:24, :])
            nc.scalar.dma_start(out=wqT_sb[:, 24:32, :], in_=wqp[:, 24:32, :])
            nc.scalar.dma_start(out=wqT_sb[:, 0:8, :], in_=wqp[:, 0:8, :])
            nc.scalar.dma_start(out=cos_sb[:, :], in_=cosk[:, :])
            nc.scalar.dma_start(out=sin_sb[:, :], in_=sink[:, :])
            nc.scalar.dma_start(out=v_sb[:, 0 : SP // 128, :], in_=pvp[:, :, :])
            nc.scalar.dma_start(out=kT_sb[:, 0:SP], in_=pkT[:, :])

            # ---- pass A: chunk-outer accumulation of K, V, Q0, Q1 (8 banks).
            # The first QDEFER chunks contribute only K/V (light, 2.1us/chunk)
            # so the PE never catches the still-warming DMA stream; their
            # Q0/Q1 contributions run as a mini-sweep at the end of the pass
            # (accumulation order within a bank is free).
            QDEFER = 8
            kps = ph1_ps.tile([128, 2, 512], F32, tag="pa0")
            vps = ph1_ps.tile([128, 2, 512], F32, tag="pa1")
            q0ps = ph1_ps.tile([128, 2, 512], F32, tag="pa2")
            q1ps = ph1_ps.tile([128, 2, 512], F32, tag="pa3")

            def q01_mms(c, st_flags):
                for g in range(2):
                    nc.tensor.matmul(
                        q0ps[:, g, :],
                        lhsT=wqT_sb[:, c, 0:128],
                        rhs=xT_sb[:, c, ts(g, 512)],
                        **st_flags,
                    )
                    nc.tensor.matmul(
                        q1ps[:, g, :],
                        lhsT=wqT_sb[:, c, ds(128, 128)],
                        rhs=xT_sb[:, c, ts(g, 512)],
                        **st_flags,
                    )

            for c in range(HCH):
                st_flags = dict(start=(c == 0), stop=(c == HCH - 1))
                for g in range(2):
                    nc.tensor.matmul(
                        kps[:, g, :],
                        lhsT=wkT_sb[:, c, :],
                        rhs=xT_sb[:, c, ts(g, 512)],
                        **st_flags,
                    )
                    nc.tensor.matmul(
                        vps[:, g, :],
                        lhsT=wvT_sb[:, c, :],
                        rhs=xT_sb[:, c, ts(g, 512)],
                        **st_flags,
                    )
                if c >= QDEFER:
                    q01_mms(c, dict(start=(c == QDEFER), stop=False))
            for c in range(QDEFER):
                q01_mms(c, dict(start=False, stop=(c == QDEFER - 1)))

            # V psum -> bf16 staging (scalar; fast consumer frees pa1)
            vt_sb = proj.tile([128, 2, 512], BF16)
            nc.scalar.activation(
                vt_sb[:, :, :], vps[:, :, :], mybir.ActivationFunctionType.Copy
            )
            # K + Q0 + Q1 ropes on DVE (overlap the transposes / pass B2)
            for g in range(2):
                _rope_write(
                    nc, rope_tmp, kT_sb[:, ds(SP + g * 512, 512)], kps[:, g, :],
                    cos_sb, sin_sb, SP + g * 512, 512,
                )
            for g in range(2):
                _rope_write(
                    nc, rope_tmp, qT[0][:, ts(g, 512)], q0ps[:, g, :],
                    cos_sb, sin_sb, SP + g * 512, 512,
                )
            for g in range(2):
                _rope_write(
                    nc, rope_tmp, qT[1][:, ts(g, 512)], q1ps[:, g, :],
                    cos_sb, sin_sb, SP + g * 512, 512,
                )
            # V transposes into v_sb chunks [SP/128 ..): reuse pa1's banks
            # ([128,128] bf16 tiles in the slot vt's copy just freed)
            for k in range(8):
                ps2 = ph1_ps.tile([128, 128], BF16, tag="pa1", name="trps")
                nc.tensor.transpose(
                    ps2[:, :], vt_sb[:, k // 4, ts(k % 4, 128)], ident[:, :]
                )
                nc.scalar.copy(v_sb[:, SP // 128 + k, :], ps2[:, :])

            # ---- passes B2/B3: Q2 (reuses K's banks), Q3
            def q_pass(j, tag):
                qps = ph1_ps.tile([128, 2, 512], F32, tag=tag, name=f"q{j}ps")
                for c in range(HCH):
                    st_flags = dict(start=(c == 0), stop=(c == HCH - 1))
                    for g in range(2):
                        nc.tensor.matmul(
                            qps[:, g, :],
                            lhsT=wqT_sb[:, c, ts(j, 128)],
                            rhs=xT_sb[:, c, ts(g, 512)],
                            **st_flags,
                        )
                return qps

            q2ps = q_pass(2, "pa0")
            for g in range(2):
                _rope_write(
                    nc, rope_tmp, qT[2][:, ts(g, 512)], q2ps[:, g, :],
                    cos_sb, sin_sb, SP + g * 512, 512,
                )
            # B3 runs g-outer so the g0 bank's accumulation finishes mid-pass;
            # its PSUM-reading rope muls (stage 1) then overlap the g1 loop,
            # freeing B3's banks (aliased by the attention pools) early.
            q3ps = ph1_ps.tile([128, 2, 512], F32, tag="pa2", name="q3ps")
            q3t, q3u = [], []
            for g in range(2):
                for c in range(HCH):
                    nc.tensor.matmul(
                        q3ps[:, g, :],
                        lhsT=wqT_sb[:, c, ts(3, 128)],
                        rhs=xT_sb[:, c, ts(g, 512)],
                        start=(c == 0),
                        stop=(c == HCH - 1),
                    )
                cs = cos_sb[:, ds(SP + g * 512, 512)]
                sn = sin_sb[:, ds(SP + g * 512, 512)]
                t = rope_tmp.tile([128, 512], F32, tag="rope_t")
                u = rope_tmp.tile([128, 512], F32, tag="rope_u")
                nc.vector.tensor_mul(t[0:64, :], q3ps[64:128, g, :], sn[0:64, :])
                nc.vector.tensor_mul(t[64:128, :], q3ps[0:64, g, :], sn[64:128, :])
                nc.vector.tensor_mul(u[:, :], q3ps[:, g, :], cs)
                q3t.append(t)
                q3u.append(u)
            for g in range(2):
                dst = qT[3][:, ts(g, 512)]
                nc.vector.tensor_sub(dst[0:64, :], q3u[g][0:64, :], q3t[g][0:64, :])
                nc.vector.tensor_add(
                    dst[64:128, :], q3u[g][64:128, :], q3t[g][64:128, :]
                )

        # ============ Phase 2+3 interleaved: attention, o_proj, RS ==========
        # separate pools: scores (2 pairs in flight), o_proj accumulators
        # (double-buffered), and single-buffered sums/att (their consumers
        # drain within the next head's lead-in) — 4+2+1+1 = 8 banks
        st_ps = ctx.enter_context(tc.tile_pool(name="st_ps", bufs=2, space="PSUM"))
        ops_ps = ctx.enter_context(tc.tile_pool(name="ops_ps", bufs=2, space="PSUM"))
        sums_ps = ctx.enter_context(
            tc.tile_pool(name="sums_ps", bufs=1, space="PSUM")
        )
        at_ps = ctx.enter_context(tc.tile_pool(name="at_ps", bufs=1, space="PSUM"))

        pt_pool = ctx.enter_context(tc.tile_pool(name="pt", bufs=6))
        pa_pool = ctx.enter_context(tc.tile_pool(name="pa", bufs=4))
        rc_pool = ctx.enter_context(tc.tile_pool(name="rc", bufs=2))
        wo_pool = ctx.enter_context(tc.tile_pool(name="wo", bufs=1))
        # deep o_proj staging: a full RS chunk (2MB) fits in SBUF so the PE
        # never stalls on part-chunk DMA writes slowed by a concurrent RS
        ob_pool = ctx.enter_context(tc.tile_pool(name="ob", bufs=16))
        wo_sb = wo_pool.tile([128, HPC, H], BF16)
        nc.scalar.dma_start(out=wo_sb[:, :, :], in_=wop[:, :, :])



        def attention_cols(off, w):
            for h in range(HPC):
                sums = sums_ps.tile([128, 512], F32, tag="sums", name=f"s{h}{off}")
                att = at_ps.tile([128, 512], F32, tag="att", name=f"a{h}{off}")
                for cc in range(KVCH // 2):
                    st = st_ps.tile([128, 2, 512], F32, tag="st", name="st")
                    pt = pt_pool.tile([128, 2, 512], BF16, name="pt")
                    for j in range(2):
                        nc.tensor.matmul(
                            st[:, j, 0:w],
                            lhsT=kT_sb[:, ts(2 * cc + j, 128)],
                            rhs=qT[h][:, ds(off, w)],
                            start=True,
                            stop=True,
                        )
                    nc.scalar.activation(
                        pt[:, :, 0:w], st[:, :, 0:w], EXP, scale=INV_SQRT_D
                    )
                    # softmax denominator: DVE pre-sums the chunk pair (bf16),
                    # halving the PE's ones-matmul work
                    padd = pa_pool.tile([128, 512], BF16, name="padd")
                    nc.vector.tensor_add(
                        padd[:, 0:w], pt[:, 0, 0:w], pt[:, 1, 0:w]
                    )
                    nc.tensor.matmul(
                        sums[:, 0:w],
                        lhsT=ones_sb[:, :],
                        rhs=padd[:, 0:w],
                        start=(cc == 0),
                        stop=(cc == KVCH // 2 - 1),
                    )
                    for j in range(2):
                        c = 2 * cc + j
                        nc.tensor.matmul(
                            att[:, 0:w],
                            lhsT=v_sb[:, c, :],
                            rhs=pt[:, j, 0:w],
                            start=(c == 0),
                            stop=(c == KVCH - 1),
                        )
                recip = rc_pool.tile([128, 512], F32, name="recip")
                nc.vector.reciprocal_approx_fast(recip[:, 0:w], sums[:, 0:w])
                nc.vector.tensor_mul(
                    attnT[h][:, ds(off, w)], att[:, 0:w], recip[:, 0:w]
                )

        def oproj_chunk(k):
            for ii in range(CHUNK_ROWS[k] // 128):
                i = CHUNK_OFF[k] // 128 + ii
                for n in range(H // 512):
                    ps = ops_ps.tile([128, 512], F32, tag="ops", name="ops")
                    ob = ob_pool.tile([128, 512], BF16, name="ob")
                    for j in range(HPC):
                        nc.tensor.matmul(
                            ps[:, :],
                            lhsT=attnT[j][:, ts(i, 128)],
                            rhs=wo_sb[:, j, ts(n, 512)],
                            start=(j == 0),
                            stop=(j == HPC - 1),
                        )
                    nc.vector.tensor_copy(ob[:, :], ps[:, :])
                    # last chunk's writes go on the scalar queue (idle by
                    # then) so they spread across descriptor queues while
                    # racing the previous chunk's ReduceScatter
                    dmaq = nc.scalar if k == NCHUNK - 1 else nc.sync
                    dmaq.dma_start(
                        out=part_chunks[k][ts(ii, 128), ts(n, 512)],
                        in_=ob[:, :],
                    )
            nc.gpsimd.collective_compute(
                "ReduceScatter",
                mybir.AluOpType.add,
                ins=[part_chunks[k][:, :].opt()],
                outs=[rs_chunks[k][:, :].opt()],
                replica_groups=[list(range(NCORES))],
            )
            # gpsimd queue: an RS-gated trigger here can't block the sync
            # queue's part writes or the scalar queue's attention exps
            nc.gpsimd.dma_start(
                out=out_ext[ds(CHUNK_OFF[k] // NCORES, CHUNK_ROWS[k] // NCORES), :],
                in_=rs_chunks[k][:, :],
            )

        # attention in four 256-wide passes, each immediately followed by its
        # o_proj chunk + ReduceScatter: every RS overlaps the next group's
        # compute, and the collective stream starts ~50us earlier than with
        # 512-wide halves.
        attention_cols(0, 256)
        oproj_chunk(0)
        attention_cols(256, 256)
        oproj_chunk(1)
        attention_cols(512, 256)
        oproj_chunk(2)
        attention_cols(768, 256)
        oproj_chunk(3)

    nc.finalize()
    return nc


def _get_nc():
    if "nc" not in _NC_CACHE:
        _NC_CACHE["nc"] = _build_nc()
    return _NC_CACHE["nc"]


def _rope_tables():
    inv_freq = 1.0 / (ROPE_THETA ** (np.arange(0, D, 2, dtype=np.float32) / D))
    pos = np.arange(KV, dtype=np.float32)
    freqs = pos[:, None] * inv_freq[None, :]  # [KV, D/2]
    emb = np.concatenate([freqs, freqs], axis=-1)  # [KV, D]
    return np.cos(emb), np.sin(emb)  # [KV, D]


def _host_rope(x, cos, sin):
    # x: [S, D]; cos/sin: [S, D]
    x1, x2 = x[:, : D // 2], x[:, D // 2 :]
    rot = np.concatenate([-x2, x1], axis=-1)
    return x * cos + rot * sin


def _pack(mat_t, inner):
    """[n*128, inner] -> [128, n, inner]: SBUF layout, partition dim first."""
    n = mat_t.shape[0] // 128
    return np.ascontiguousarray(mat_t.reshape(n, 128, inner).transpose(1, 0, 2))


def kernel(hidden_states, past_k, past_v, Wq, Wk, Wv, Wo, trace=False):
    global LAST_RESULT
    bf = ml_dtypes.bfloat16
    x = np.asarray(hidden_states, dtype=np.float32)[0]  # [SQ, H]
    xTp = _pack(np.ascontiguousarray(x.T), SQ).astype(bf)
    cos, sin = _rope_tables()  # [KV, D] f32
    cosT = np.ascontiguousarray(cos.T).astype(bf)
    sinT = np.ascontiguousarray(sin.T).astype(bf)

    in_maps = []
    for m in range(NCORES):
        qr = slice(m * DQ, (m + 1) * DQ)
        kr = slice(m * D, (m + 1) * D)
        in_maps.append(
            {
                "xTp": xTp,
                "wqp": _pack(np.asarray(Wq)[qr].T, DQ).astype(bf),
                "wkp": _pack(np.asarray(Wk)[kr].T, D).astype(bf),
                "wvp": _pack(np.asarray(Wv)[kr].T, D).astype(bf),
                "wop": _pack(np.asarray(Wo)[:, qr].T, H).astype(bf),
                "pkT": np.ascontiguousarray(
                    _host_rope(
                        np.asarray(past_k, dtype=np.float32)[0, m], cos[:SP], sin[:SP]
                    ).T
                ).astype(bf),
                "pvp": _pack(np.asarray(past_v)[0, m], D).astype(bf),
                "cosk": cosT,
                "sink": sinT,
            }
        )

    nc = _get_nc()
    res = run_bass_kernel_spmd(
        nc, in_maps, core_ids=list(range(NCORES)), trace=trace
    )
    LAST_RESULT = res
    # Each core's "out" holds NCHUNK blocks of CHUNK_ROWS[k]/8 rows; block k
    # of core m is global rows CHUNK_OFF[k] + rsh_k*[m, m+1).
    out = np.empty((SQ, H), dtype=np.float32)
    for m in range(NCORES):
        shard = np.asarray(res.results[m]["out"], dtype=np.float32)
        for k in range(NCHUNK):
            rsh = CHUNK_ROWS[k] // NCORES
            soff = CHUNK_OFF[k] // NCORES
            out[CHUNK_OFF[k] + rsh * m : CHUNK_OFF[k] + rsh * (m + 1)] = shard[
                soff : soff + rsh
            ]
    return out.reshape(B, SQ, H)


# revision 67
# speedup vs baseline: 1.1431x; 1.0075x over previous
"""Tensor-parallel GQA attention block (AtlasAttentionWrapper) on 8 TRN2 cores.

Sharding: TP over heads. Core m owns query heads [4m..4m+3] (Wq rows
m*512:(m+1)*512), KV head m (Wk/Wv rows m*128:(m+1)*128, past_k/past_v head m)
and Wo columns m*512:(m+1)*512. Each core computes a full [1024, 4096] o_proj
partial; chunked ReduceScatters ([512, 256, 256] rows, pipelined under the
remaining attention/o_proj compute) leave each core 1/8 of the rows of each
chunk; the host reassembles.

All device inputs are host-packed into SBUF layout [128, chunks, inner] so
every DMA is a large transfer with long contiguous per-partition lines.

Phase 1 runs chunk-outer with persistent PSUM accumulators so the PE starts
as soon as the first weight/xT pieces land:
  pass A  accumulates K, V, Q0 (6 banks) over all 32 contraction chunks,
  passes B1..B3 accumulate Q1..Q3 (2 banks each), reusing banks whose
  consumers (V copy, K rope, Q0 rope) have already drained.
Each rope is emitted right after its producing pass so DVE overlaps the next
pass's matmuls; per-head qT tiles keep attention from waiting on later ropes.
Attention (scores built transposed, exp on scalar with 2-chunk batching,
softmax denominator via ones-matmul) and o_proj + chunked ReduceScatter are
interleaved: attn(g0), oproj k0/k1 + RS, attn(g1), oproj k2/k3 + RS.
"""

import sys

if "/opt/trn_rl_repo" not in sys.path:
    sys.path.insert(0, "/opt/trn_rl_repo")

from contextlib import ExitStack

import ml_dtypes
import numpy as np

import concourse.bass as bass
import concourse.tile as tile
from concourse import bacc, mybir
from concourse.bass import ds, ts
from concourse.bass_utils import run_bass_kernel_spmd
from concourse.masks import make_identity

NCORES = 8
B, SQ, H = 1, 1024, 4096
NH, NKV, D = 32, 8, 128
SP = 1024
KV = SP + SQ  # 2048
HPC = NH // NCORES  # 4 query heads per core
DQ = HPC * D  # 512
SH = SQ // NCORES  # 128 output rows per core after ReduceScatter
ROPE_THETA = 10000.0
INV_SQRT_D = 1.0 / float(np.sqrt(D))

BF16 = mybir.dt.bfloat16
F32 = mybir.dt.float32
HCH = H // 128  # 32 contraction chunks
KVCH = KV // 128  # 16 kv chunks
# ReduceScatter chunk sizes (rows), matching the attention column groups:
# early 2MB chunks start the (HBM-bound, continuously-busy) collective
# stream as soon as possible; the final narrow group + 1MB chunk pulls the
# last, fully-exposed RS forward.
CHUNK_ROWS = [256, 256, 256, 256]
CHUNK_OFF = [0, 256, 512, 768]
NCHUNK = len(CHUNK_ROWS)
EXP = mybir.ActivationFunctionType.Exp

LAST_RESULT = None
_NC_CACHE = {}


def _rope_write(nc, tmp_pool, dst, src, cos_sb, sin_sb, pos, width):
    """dst[d, s] = rope(src)[d, s] for s in [pos, pos+width) absolute positions.

    src: AP [128, width] (PSUM f32 or SBUF bf16), dst: SBUF bf16 AP.
    rope: out[d<64] = x[d]*cos[d] - x[d+64]*sin[d]
          out[d>=64] = x[d]*cos[d] + x[d-64]*sin[d]
    """
    cs = cos_sb[:, ds(pos, width)]
    sn = sin_sb[:, ds(pos, width)]
    t = tmp_pool.tile([128, width], F32, tag="rope_t")
    u = tmp_pool.tile([128, width], F32, tag="rope_u")
    nc.vector.tensor_mul(t[0:64, :], src[64:128, :], sn[0:64, :])
    nc.vector.tensor_mul(t[64:128, :], src[0:64, :], sn[64:128, :])
    nc.vector.tensor_mul(u[:, :], src[:, :], cs)
    nc.vector.tensor_sub(dst[0:64, :], u[0:64, :], t[0:64, :])
    nc.vector.tensor_add(dst[64:128, :], u[64:128, :], t[64:128, :])


def _build_nc():
    nc = bacc.Bacc(None, target_bir_lowering=False, debug=False)

    xTp = nc.declare_dram_parameter("xTp", [128, HCH, SQ], BF16, False)
    wqp = nc.declare_dram_parameter("wqp", [128, HCH, DQ], BF16, False)
    wkp = nc.declare_dram_parameter("wkp", [128, HCH, D], BF16, False)
    wvp = nc.declare_dram_parameter("wvp", [128, HCH, D], BF16, False)
    wop = nc.declare_dram_parameter("wop", [128, HPC, H], BF16, False)
    pkT = nc.declare_dram_parameter("pkT", [D, SP], BF16, False)
    pvp = nc.declare_dram_parameter("pvp", [128, SP // 128, D], BF16, False)
    cosk = nc.declare_dram_parameter("cosk", [D, KV], BF16, False)
    sink = nc.declare_dram_parameter("sink", [D, KV], BF16, False)
    out_ext = nc.declare_dram_parameter("out", [SH, H], BF16, True)

    with tile.TileContext(nc) as tc, ExitStack() as ctx:
        # ---- persistent SBUF residents (live across all phases)
        const = ctx.enter_context(tc.tile_pool(name="const", bufs=1))
        kT_sb = const.tile([128, KV], BF16)  # roped K^T  [d, kv]
        v_sb = const.tile([128, KVCH, D], BF16)  # V chunks [kv%128, chunk, d]
        # per-head roped Q^T / attn^T tiles (separate tiles keep readers from
        # waiting on later heads' writes)
        qT = [const.tile([128, SQ], BF16, name=f"qT{j}") for j in range(HPC)]
        attnT = [const.tile([128, SQ], BF16, name=f"attnT{j}") for j in range(HPC)]
        cos_sb = const.tile([128, KV], BF16)
        sin_sb = const.tile([128, KV], BF16)
        ident = const.tile([128, 128], BF16)
        ones_sb = const.tile([128, 128], BF16)

        make_identity(nc, ident[:, :])
        nc.vector.memset(ones_sb[:, :], 1.0)

        rope_tmp = ctx.enter_context(tc.tile_pool(name="rope_tmp", bufs=2))
        dram = ctx.enter_context(tc.tile_pool(name="dram", bufs=1, space="DRAM"))
        part_chunks = []
        rs_chunks = []
        for k in range(NCHUNK):
            part_chunks.append(
                dram.tile([CHUNK_ROWS[k], H], BF16, tag=f"part{k}", name=f"part{k}")
            )
            rs_chunks.append(
                dram.tile(
                    [CHUNK_ROWS[k] // NCORES, H], BF16, tag=f"rs{k}", name=f"rs{k}"
                )
            )

        # ================= Phase 1: projections + rope ==================
        with tc.tile_pool(name="proj", bufs=1) as proj, tc.tile_pool(
            name="ph1_ps", bufs=1, space="PSUM"
        ) as ph1_ps:
            xT_sb = proj.tile([128, HCH, SQ], BF16)
            wqT_sb = proj.tile([128, HCH, DQ], BF16)
            wkT_sb = proj.tile([128, HCH, D], BF16)
            wvT_sb = proj.tile([128, HCH, D], BF16)

            # Weight/xT stream: small first pieces so the chunk-outer pass A
            # starts within a few us; later pieces sized to stay ahead of PE.
            nc.sync.dma_start(out=wkT_sb[:, 0:2, :], in_=wkp[:, 0:2, :])
            nc.sync.dma_start(out=wvT_sb[:, 0:2, :], in_=wvp[:, 0:2, :])
            nc.sync.dma_start(out=xT_sb[:, 0:1, :], in_=xTp[:, 0:1, :])
            nc.sync.dma_start(out=wkT_sb[:, 2:8, :], in_=wkp[:, 2:8, :])
            nc.sync.dma_start(out=wvT_sb[:, 2:8, :], in_=wvp[:, 2:8, :])
            nc.sync.dma_start(out=xT_sb[:, 1:2, :], in_=xTp[:, 1:2, :])
            nc.sync.dma_start(out=xT_sb[:, 2:4, :], in_=xTp[:, 2:4, :])
            nc.sync.dma_start(out=xT_sb[:, 4:6, :], in_=xTp[:, 4:6, :])
            nc.sync.dma_start(out=wkT_sb[:, 8:16, :], in_=wkp[:, 8:16, :])
            nc.sync.dma_start(out=wvT_sb[:, 8:16, :], in_=wvp[:, 8:16, :])
            nc.sync.dma_start(out=xT_sb[:, 6:8, :], in_=xTp[:, 6:8, :])
            nc.sync.dma_start(out=xT_sb[:, 8:12, :], in_=xTp[:, 8:12, :])
            nc.sync.dma_start(out=xT_sb[:, 12:16, :], in_=xTp[:, 12:16, :])
            nc.sync.dma_start(out=wkT_sb[:, 16:32, :], in_=wkp[:, 16:32, :])
            nc.sync.dma_start(out=wvT_sb[:, 16:32, :], in_=wvp[:, 16:32, :])
            nc.sync.dma_start(out=xT_sb[:, 16:20, :], in_=xTp[:, 16:20, :])
            nc.sync.dma_start(out=xT_sb[:, 20:24, :], in_=xTp[:, 20:24, :])
            nc.sync.dma_start(out=xT_sb[:, 24:32, :], in_=xTp[:, 24:32, :])
            # scalar queue: Q weights in consumption order (chunks >= QDEFER
            # are needed first; chunks 0:8 only at the end-of-A mini-sweep),
            # then the small rope/past tensors (first needed at the K rope).
            nc.scalar.dma_start(out=wqT_sb[:, 8:16, :], in_=wqp[:, 8:16, :])
            nc.scalar.dma_start(out=wqT_sb[:, 16:24, :], in_=wqp[:, 16:24, :])
            nc.scalar.dma_start(out=wqT_sb[:, 24:32, :], in_=wqp[:, 24:32, :])
            nc.scalar.dma_start(out=wqT_sb[:, 0:8, :], in_=wqp[:, 0:8, :])
            nc.scalar.dma_start(out=cos_sb[:, :], in_=cosk[:, :])
            nc.scalar.dma_start(out=sin_sb[:, :], in_=sink[:, :])
            nc.scalar.dma_start(out=v_sb[:, 0 : SP // 128, :], in_=pvp[:, :, :])
            nc.scalar.dma_start(out=kT_sb[:, 0:SP], in_=pkT[:, :])

            # ---- pass A: chunk-outer accumulation of K, V, Q0, Q1 (8 banks).
            # The first QDEFER chunks contribute only K/V (light, 2.1us/chunk)
            # so the PE never catches the still-warming DMA stream; their
            # Q0/Q1 contributions run as a mini-sweep at the end of the pass
            # (accumulation order within a bank is free).
            QDEFER = 8
            kps = ph1_ps.tile([128, 2, 512], F32, tag="pa0")
            vps = ph1_ps.tile([128, 2, 512], F32, tag="pa1")
            q0ps = ph1_ps.tile([128, 2, 512], F32, tag="pa2")
            q1ps = ph1_ps.tile([128, 2, 512], F32, tag="pa3")

            def q01_mms(c, st_flags):
                for g in range(2):
                    nc.tensor.matmul(
                        q0ps[:, g, :],
                        lhsT=wqT_sb[:, c, 0:128],
                        rhs=xT_sb[:, c, ts(g, 512)],
                        **st_flags,
                    )
                    nc.tensor.matmul(
                        q1ps[:, g, :],
                        lhsT=wqT_sb[:, c, ds(128, 128)],
                        rhs=xT_sb[:, c, ts(g, 512)],
                        **st_flags,
                    )

            for c in range(HCH):
                st_flags = dict(start=(c == 0), stop=(c == HCH - 1))
                for g in range(2):
                    nc.tensor.matmul(
                        kps[:, g, :],
                        lhsT=wkT_sb[:, c, :],
                        rhs=xT_sb[:, c, ts(g, 512)],
                        **st_flags,
                    )
                    nc.tensor.matmul(
                        vps[:, g, :],
                        lhsT=wvT_sb[:, c, :],
                        rhs=xT_sb[:, c, ts(g, 512)],
                        **st_flags,
                    )
                if c >= QDEFER:
                    q01_mms(c, dict(start=(c == QDEFER), stop=False))
            for c in range(QDEFER):
                q01_mms(c, dict(start=False, stop=(c == QDEFER - 1)))

            # V psum -> bf16 staging (scalar; fast consumer frees pa1)
            vt_sb = proj.tile([128, 2, 512], BF16)
            nc.scalar.activation(
                vt_sb[:, :, :], vps[:, :, :], mybir.ActivationFunctionType.Copy
            )
            # K + Q0 + Q1 ropes on DVE (overlap the transposes / pass B2)
            for g in range(2):
                _rope_write(
                    nc, rope_tmp, kT_sb[:, ds(SP + g * 512, 512)], kps[:, g, :],
                    cos_sb, sin_sb, SP + g * 512, 512,
                )
            for g in range(2):
                _rope_write(
                    nc, rope_tmp, qT[0][:, ts(g, 512)], q0ps[:, g, :],
                    cos_sb, sin_sb, SP + g * 512, 512,
                )
            for g in range(2):
                _rope_write(
                    nc, rope_tmp, qT[1][:, ts(g, 512)], q1ps[:, g, :],
                    cos_sb, sin_sb, SP + g * 512, 512,
                )
            # V transposes into v_sb chunks [SP/128 ..): reuse pa1's banks
            # ([128,128] bf16 tiles in the slot vt's copy just freed)
            for k in range(8):
                ps2 = ph1_ps.tile([128, 128], BF16, tag="pa1", name="trps")
                nc.tensor.transpose(
                    ps2[:, :], vt_sb[:, k // 4, ts(k % 4, 128)], ident[:, :]
                )
                nc.scalar.copy(v_sb[:, SP // 128 + k, :], ps2[:, :])

            # ---- passes B2/B3: Q2 (reuses K's banks), Q3
            def q_pass(j, tag):
                qps = ph1_ps.tile([128, 2, 512], F32, tag=tag, name=f"q{j}ps")
                for c in range(HCH):
                    st_flags = dict(start=(c == 0), stop=(c == HCH - 1))
                    for g in range(2):
                        nc.tensor.matmul(
                            qps[:, g, :],
                            lhsT=wqT_sb[:, c, ts(j, 128)],
                            rhs=xT_sb[:, c, ts(g, 512)],
                            **st_flags,
                        )
                return qps

            q2ps = q_pass(2, "pa0")
            for g in range(2):
                _rope_write(
                    nc, rope_tmp, qT[2][:, ts(g, 512)], q2ps[:, g, :],
                    cos_sb, sin_sb, SP + g * 512, 512,
                )
            # B3 runs g-outer so the g0 bank's accumulation finishes mid-pass;
            # its PSUM-reading rope muls (stage 1) then overlap the g1 loop,
            # freeing B3's banks (aliased by the attention pools) early.
            q3ps = ph1_ps.tile([128, 2, 512], F32, tag="pa2", name="q3ps")
            q3t, q3u = [], []
            for g in range(2):
                for c in range(HCH):
                    nc.tensor.matmul(
                        q3ps[:, g, :],
                        lhsT=wqT_sb[:, c, ts(3, 128)],
                        rhs=xT_sb[:, c, ts(g, 512)],
                        start=(c == 0),
                        stop=(c == HCH - 1),
                    )
                cs = cos_sb[:, ds(SP + g * 512, 512)]
                sn = sin_sb[:, ds(SP + g * 512, 512)]
                t = rope_tmp.tile([128, 512], F32, tag="rope_t")
                u = rope_tmp.tile([128, 512], F32, tag="rope_u")
                nc.vector.tensor_mul(t[0:64, :], q3ps[64:128, g, :], sn[0:64, :])
                nc.vector.tensor_mul(t[64:128, :], q3ps[0:64, g, :], sn[64:128, :])
                nc.vector.tensor_mul(u[:, :], q3ps[:, g, :], cs)
                q3t.append(t)
                q3u.append(u)
            for g in range(2):
                dst = qT[3][:, ts(g, 512)]
                nc.vector.tensor_sub(dst[0:64, :], q3u[g][0:64, :], q3t[g][0:64, :])
                nc.vector.tensor_add(
                    dst[64:128, :], q3u[g][64:128, :], q3t[g][64:128, :]
                )

        # ============ Phase 2+3 interleaved: attention, o_proj, RS ==========
        # separate pools: scores (2 pairs in flight), o_proj accumulators
        # (double-buffered), and single-buffered sums/att (their consumers
        # drain within the next head's lead-in) — 4+2+1+1 = 8 banks
        st_ps = ctx.enter_context(tc.tile_pool(name="st_ps", bufs=2, space="PSUM"))
        ops_ps = ctx.enter_context(tc.tile_pool(name="ops_ps", bufs=2, space="PSUM"))
        sums_ps = ctx.enter_context(
            tc.tile_pool(name="sums_ps", bufs=1, space="PSUM")
        )
        at_ps = ctx.enter_context(tc.tile_pool(name="at_ps", bufs=1, space="PSUM"))

        pt_pool = ctx.enter_context(tc.tile_pool(name="pt", bufs=6))
        pa_pool = ctx.enter_context(tc.tile_pool(name="pa", bufs=4))
        rc_pool = ctx.enter_context(tc.tile_pool(name="rc", bufs=2))
        wo_pool = ctx.enter_context(tc.tile_pool(name="wo", bufs=1))
        # deep o_proj staging: a full RS chunk (2MB) fits in SBUF so the PE
        # never stalls on part-chunk DMA writes slowed by a concurrent RS
        ob_pool = ctx.enter_context(tc.tile_pool(name="ob", bufs=16))
        wo_sb = wo_pool.tile([128, HPC, H], BF16)
        nc.scalar.dma_start(out=wo_sb[:, :, :], in_=wop[:, :, :])



        def attention_cols(off, w):
            for h in range(HPC):
                sums = sums_ps.tile([128, 512], F32, tag="sums", name=f"s{h}{off}")
                att = at_ps.tile([128, 512], F32, tag="att", name=f"a{h}{off}")
                for cc in range(KVCH // 2):
                    st = st_ps.tile([128, 2, 512], F32, tag="st", name="st")
                    pt = pt_pool.tile([128, 2, 512], BF16, name="pt")
                    for j in range(2):
                        nc.tensor.matmul(
                            st[:, j, 0:w],
                            lhsT=kT_sb[:, ts(2 * cc + j, 128)],
                            rhs=qT[h][:, ds(off, w)],
                            start=True,
                            stop=True,
                        )
                    nc.scalar.activation(
                        pt[:, :, 0:w], st[:, :, 0:w], EXP, scale=INV_SQRT_D
                    )
                    # softmax denominator: DVE pre-sums the chunk pair (bf16),
                    # halving the PE's ones-matmul work
                    padd = pa_pool.tile([128, 512], BF16, name="padd")
                    nc.vector.tensor_add(
                        padd[:, 0:w], pt[:, 0, 0:w], pt[:, 1, 0:w]
                    )
                    nc.tensor.matmul(
                        sums[:, 0:w],
                        lhsT=ones_sb[:, :],
                        rhs=padd[:, 0:w],
                        start=(cc == 0),
                        stop=(cc == KVCH // 2 - 1),
                    )
                    for j in range(2):
                        c = 2 * cc + j
                        nc.tensor.matmul(
                            att[:, 0:w],
                            lhsT=v_sb[:, c, :],
                            rhs=pt[:, j, 0:w],
                            start=(c == 0),
                            stop=(c == KVCH - 1),
                        )
                recip = rc_pool.tile([128, 512], F32, name="recip")
                nc.vector.reciprocal_approx_fast(recip[:, 0:w], sums[:, 0:w])
                nc.vector.tensor_mul(
                    attnT[h][:, ds(off, w)], att[:, 0:w], recip[:, 0:w]
                )

        def oproj_chunk(k):
            for ii in range(CHUNK_ROWS[k] // 128):
                i = CHUNK_OFF[k] // 128 + ii
                for n in range(H // 512):
                    ps = ops_ps.tile([128, 512], F32, tag="ops", name="ops")
                    ob = ob_pool.tile([128, 512], BF16, name="ob")
                    for j in range(HPC):
                        nc.tensor.matmul(
                            ps[:, :],
                            lhsT=attnT[j][:, ts(i, 128)],
                            rhs=wo_sb[:, j, ts(n, 512)],
                            start=(j == 0),
                            stop=(j == HPC - 1),
                        )
                    nc.vector.tensor_copy(ob[:, :], ps[:, :])
                    # last chunk's writes go on the scalar queue (idle by
                    # then) so they spread across descriptor queues while
                    # racing the previous chunk's ReduceScatter
                    dmaq = nc.scalar if k == NCHUNK - 1 else nc.sync
                    dmaq.dma_start(
                        out=part_chunks[k][ts(ii, 128), ts(n, 512)],
                        in_=ob[:, :],
                    )
            nc.gpsimd.collective_compute(
                "ReduceScatter",
                mybir.AluOpType.add,
                ins=[part_chunks[k][:, :].opt()],
                outs=[rs_chunks[k][:, :].opt()],
                replica_groups=[list(range(NCORES))],
            )
            # gpsimd queue: an RS-gated trigger here can't block the sync
            # queue's part writes or the scalar queue's attention exps
            nc.gpsimd.dma_start(
                out=out_ext[ds(CHUNK_OFF[k] // NCORES, CHUNK_ROWS[k] // NCORES), :],
                in_=rs_chunks[k][:, :],
            )

        # attention in four 256-wide passes, each immediately followed by its
        # o_proj chunk + ReduceScatter: every RS overlaps the next group's
        # compute, and the collective stream starts ~50us earlier than with
        # 512-wide halves.
        attention_cols(0, 256)
        oproj_chunk(0)
        attention_cols(256, 256)
        oproj_chunk(1)
        attention_cols(512, 256)
        oproj_chunk(2)
        attention_cols(768, 256)
        oproj_chunk(3)

    nc.finalize()
    return nc


def _get_nc():
    if "nc" not in _NC_CACHE:
        _NC_CACHE["nc"] = _build_nc()
    return _NC_CACHE["nc"]


def _rope_tables():
    inv_freq = 1.0 / (ROPE_THETA ** (np.arange(0, D, 2, dtype=np.float32) / D))
    pos = np.arange(KV, dtype=np.float32)
    freqs = pos[:, None] * inv_freq[None, :]  # [KV, D/2]
    emb = np.concatenate([freqs, freqs], axis=-1)  # [KV, D]
    return np.cos(emb), np.sin(emb)  # [KV, D]


def _host_rope(x, cos, sin):
    # x: [S, D]; cos/sin: [S, D]
    x1, x2 = x[:, : D // 2], x[:, D // 2 :]
    rot = np.concatenate([-x2, x1], axis=-1)
    return x * cos + rot * sin


def _pack(mat_t, inner):
    """[n*128, inner] -> [128, n, inner]: SBUF layout, partition dim first."""
    n = mat_t.shape[0] // 128
    return np.ascontiguousarray(mat_t.reshape(n, 128, inner).transpose(1, 0, 2))


def kernel(hidden_states, past_k, past_v, Wq, Wk, Wv, Wo, trace=False):
    global LAST_RESULT
    bf = ml_dtypes.bfloat16
    x = np.asarray(hidden_states, dtype=np.float32)[0]  # [SQ, H]
    xTp = _pack(np.ascontiguousarray(x.T), SQ).astype(bf)
    cos, sin = _rope_tables()  # [KV, D] f32
    cosT = np.ascontiguousarray(cos.T).astype(bf)
    sinT = np.ascontiguousarray(sin.T).astype(bf)

    in_maps = []
    for m in range(NCORES):
        qr = slice(m * DQ, (m + 1) * DQ)
        kr = slice(m * D, (m + 1) * D)
        in_maps.append(
            {
                "xTp": xTp,
                "wqp": _pack(np.asarray(Wq)[qr].T, DQ).astype(bf),
                "wkp": _pack(np.asarray(Wk)[kr].T, D).astype(bf),
                "wvp": _pack(np.asarray(Wv)[kr].T, D).astype(bf),
                "wop": _pack(np.asarray(Wo)[:, qr].T, H).astype(bf),
                "pkT": np.ascontiguousarray(
                    _host_rope(
                        np.asarray(past_k, dtype=np.float32)[0, m], cos[:SP], sin[:SP]
                    ).T
                ).astype(bf),
                "pvp": _pack(np.asarray(past_v)[0, m], D).astype(bf),
                "cosk": cosT,
                "sink": sinT,
            }
        )

    nc = _get_nc()
    res = run_bass_kernel_spmd(
        nc, in_maps, core_ids=list(range(NCORES)), trace=trace
    )
    LAST_RESULT = res
    # Each core's "out" holds NCHUNK blocks of CHUNK_ROWS[k]/8 rows; block k
    # of core m is global rows CHUNK_OFF[k] + rsh_k*[m, m+1).
    out = np.empty((SQ, H), dtype=np.float32)
    for m in range(NCORES):
        shard = np.asarray(res.results[m]["out"], dtype=np.float32)
        for k in range(NCHUNK):
            rsh = CHUNK_ROWS[k] // NCORES
            soff = CHUNK_OFF[k] // NCORES
            out[CHUNK_OFF[k] + rsh * m : CHUNK_OFF[k] + rsh * (m + 1)] = shard[
                soff : soff + rsh
            ]
    return out.reshape(B, SQ, H)
